# revision 1
# baseline (speedup 1.0000x reference)
"""Trainium2 Bass kernel for 3-level hierarchical hypergraph GNN (HGNN).

Strategy (8 NeuronCores, one SPMD NEFF, per-core index data):
  - Nodes of every level sharded round-robin: global id g -> core g%8, slot g//8.
  - Incidence entries assigned to the owner core of their node endpoint.
  - L_apply = two segment-sum passes:
      stage1 (edges): partial edge sums via dma_gather(node rows) + weighted
                      one-hot matmul into 128-row PSUM edge windows; AllReduce.
      stage2 (nodes): gather full-edge-table rows + one-hot matmul into local
                      node windows (complete rows, no reduction needed).
    Degree scalings (Dv^-1/2, De^-1) folded into per-entry weights.
  - Pools: same segment-sum, targets = remapped cluster rows; ReduceScatter
    leaves each core exactly its local cluster slice.
  - Unpool: dma_gather from AllGathered coarse tables.
  - Linears: per-128-row-chunk PE transpose + matmul; concat = two
    accumulating matmuls; bias via broadcast-tile add.
"""
import sys

sys.path.insert(0, "/opt/trn_rl_repo")
import numpy as np

C = 8
CH = 32  # gather chunk size in 128-entry blocks

N0, N1, N2 = 100000, 25000, 6250
E0, E1, E2 = 20000, 5000, 1250
D_IN, D_H, D_OUT = 128, 128, 64


def _pad128(n):
    return ((n + 127) // 128) * 128


def _pad_local(n):
    return _pad128(-(-n // C))


# ---------------------------------------------------------------- host side
def _degree_weights(vi, ei, n, e):
    ones = np.ones(len(vi), np.float32)
    dV = np.bincount(vi, weights=ones, minlength=n)
    dE = np.bincount(ei, weights=ones, minlength=e)
    dv_is = np.where(dV > 0, dV ** -0.5, 0.0).astype(np.float32)
    de_i = np.where(dE > 0, 1.0 / dE, 0.0).astype(np.float32)
    return dv_is, de_i


def _plane_idx(idx):
    """int array (L,) -> [128, L//16] int16 (16-partition wrap, replicated x8)."""
    assert len(idx) % 16 == 0
    assert idx.max(initial=0) < 32768
    return np.tile(idx.astype(np.int16).reshape(-1, 16).T, (C, 1)).copy()


def _plane_tw(tgt, wgt):
    """-> [128, 2*B] f32, columns (2b, 2b+1) = (target, weight) of block b."""
    nb = len(tgt) // 128
    out = np.empty((128, 2 * nb), np.float32)
    out[:, 0::2] = tgt.astype(np.float32).reshape(nb, 128).T
    out[:, 1::2] = wgt.astype(np.float32).reshape(nb, 128).T
    return out


class SegStage:
    """Host data for one segment-sum stage, uniform structure across cores."""

    def __init__(self, name, gidx, trow, wgt, n_rows_padded):
        self.name = name
        self.nw = n_rows_padded // 128
        cnts = np.stack([
            np.bincount(trow[c] // 128, minlength=self.nw) for c in range(C)
        ])
        self.bpw = np.maximum(1, -(-cnts.max(axis=0) // 128)).astype(np.int64)
        self.nblocks = int(self.bpw.sum())
        L = 128 * self.nblocks
        self.L = L
        self.idx_planes, self.tw_planes = [], []
        starts = np.concatenate([[0], np.cumsum(self.bpw[:-1])]) * 128
        for c in range(C):
            order = np.argsort(trow[c] // 128, kind="stable")
            gi = np.zeros(L, np.int64)
            tg = np.zeros(L, np.int64)
            wg = np.zeros(L, np.float32)
            w_of = trow[c] // 128
            pos = np.searchsorted(w_of[order], np.arange(self.nw))
            end = np.searchsorted(w_of[order], np.arange(self.nw), side="right")
            for w in range(self.nw):
                sel = order[pos[w]:end[w]]
                s = starts[w]
                gi[s:s + len(sel)] = gidx[c][sel]
                tg[s:s + len(sel)] = trow[c][sel] % 128
                wg[s:s + len(sel)] = wgt[c][sel]
            self.idx_planes.append(_plane_idx(gi))
            self.tw_planes.append(_plane_tw(tg, wg))

    def renamed(self, name):
        st = SegStage.__new__(SegStage)
        st.__dict__ = dict(self.__dict__)
        st.name = name
        return st


def _gather_planes(idx_per_core):
    """Plain gather streams (no reduction), padded to a 128 multiple."""
    L = _pad128(len(idx_per_core[0]))
    planes = []
    for c in range(C):
        gi = np.zeros(L, np.int64)
        gi[: len(idx_per_core[c])] = idx_per_core[c]
        planes.append(_plane_idx(gi))
    return L, planes


# ---------------------------------------------------------------- device side
class Builder:
    def __init__(self, nc, mybir):
        self.nc = nc
        self.mybir = mybir
        self.tc = None
        self.inputs = {}  # name -> per-core list of arrays (or one shared array)

    def add_input(self, name, shape, dtype, arrays):
        assert name not in self.inputs, name
        t = self.nc.dram_tensor(name, list(shape), dtype, kind="ExternalInput")
        self.inputs[name] = arrays
        return t

    def setup_pools(self, ctx):
        tc = self.tc
        self.p_const = ctx.enter_context(tc.tile_pool(name="const", bufs=1))
        self.p_gath = ctx.enter_context(tc.tile_pool(name="gath", bufs=3))
        self.p_meta = ctx.enter_context(tc.tile_pool(name="meta", bufs=3))
        self.p_oh = ctx.enter_context(tc.tile_pool(name="oh", bufs=4))
        self.p_fl = ctx.enter_context(tc.tile_pool(name="fl", bufs=4))
        self.p_lin = ctx.enter_context(tc.tile_pool(name="lin", bufs=3))
        self.p_ps = ctx.enter_context(tc.tile_pool(name="ps", bufs=2, space="PSUM"))
        self.p_ps2 = ctx.enter_context(tc.tile_pool(name="ps2", bufs=2, space="PSUM"))

    def setup_consts(self):
        f32 = self.mybir.dt.float32
        iota = np.tile(np.arange(128, dtype=np.float32), (128, 1))
        ident = np.eye(128, dtype=np.float32)
        self.iota_t = self.const_mat("c_iota", iota)
        self.ident_t = self.const_mat("c_ident", ident)

    def const_mat(self, name, arr):
        f32 = self.mybir.dt.float32
        arr = np.ascontiguousarray(arr, np.float32)
        d = self.add_input(name, list(arr.shape), f32, arr)
        t = self.p_const.tile(list(arr.shape), f32, tag=name)
        self.nc.sync.dma_start(t[:], d[:, :])
        return t

    def emit_seg(self, st: SegStage, src_dram, dst_dram, D, out_op):
        """One segment-sum stage. out_op in ('copy', 'relu')."""
        nc, mybir = self.nc, self.mybir
        f32, i16 = mybir.dt.float32, mybir.dt.int16
        idx_d = self.add_input(f"{st.name}_idx", [128, st.L // 16], i16,
                               st.idx_planes)
        tw_d = self.add_input(f"{st.name}_tw", [128, 2 * st.nblocks], f32,
                              st.tw_planes)

        sched = []  # block -> (window, j, is_last)
        for w in range(st.nw):
            for j in range(st.bpw[w]):
                sched.append((w, j, j == st.bpw[w] - 1))

        b = 0
        ps = None
        for start in range(0, st.nblocks, CH):
            nb = min(CH, st.nblocks - start)
            idx_t = self.p_meta.tile([128, nb * 8], i16, tag="idx")
            nc.sync.dma_start(idx_t[:],
                              idx_d[:, start * 8:(start + nb) * 8])
            tw_t = self.p_meta.tile([128, 2 * nb], f32, tag="tw")
            nc.sync.dma_start(tw_t[:],
                              tw_d[:, 2 * start:2 * (start + nb)])
            g_t = self.p_gath.tile([128, nb, D], f32, tag=f"g{D}")
            nc.gpsimd.dma_gather(
                g_t[:], src_dram[:, :], idx_t[:],
                num_idxs=nb * 128, num_idxs_reg=nb * 128, elem_size=D,
                single_packet=False)
            for k in range(nb):
                w, j, last = sched[b]
                if j == 0:
                    ps = self.p_ps.tile([128, D], f32, tag="seg")
                oh = self.p_oh.tile([128, 128], f32, tag="oh")
                nc.vector.tensor_scalar(
                    oh[:], self.iota_t[:],
                    tw_t[:, 2 * k:2 * k + 1], tw_t[:, 2 * k + 1:2 * k + 2],
                    mybir.AluOpType.is_equal, mybir.AluOpType.mult)
                nc.tensor.matmul(ps[:], oh[:], g_t[:, k, :],
                                 start=(j == 0), stop=last)
                if last:
                    r = self.p_fl.tile([128, D], f32, tag="fl")
                    if out_op == "relu":
                        nc.vector.tensor_scalar_max(r[:], ps[:], 0.0)
                    else:
                        nc.vector.tensor_copy(r[:], ps[:])
                    nc.sync.dma_start(dst_dram[128 * w:128 * (w + 1), :], r[:])
                b += 1

    def emit_linear(self, name, sources, Ws, bias_t, dst_dram, nchunks, Dout,
                    D=128):
        """dst chunk = sum_s source_s_chunk @ Ws[s] + bias.

        sources: list of (src_dram, None) for sequential 128-row chunks, or
        (src_dram, idx_dram) for rows gathered via a per-core index stream."""
        nc, mybir = self.nc, self.mybir
        f32, i16 = mybir.dt.float32, mybir.dt.int16
        GCH = 16  # chunks per gather group
        gtiles = {}

        def gathered_view(si, i, src_dram, idx_dram):
            grp = i // GCH
            if (si, grp) not in gtiles:
                n_in = min(GCH, nchunks - grp * GCH)
                idx_t = self.p_meta.tile([128, n_in * 8], i16, tag="lidx")
                nc.sync.dma_start(
                    idx_t[:],
                    idx_dram[:, grp * GCH * 8:(grp * GCH + n_in) * 8])
                g_t = self.p_gath.tile([128, n_in, D], f32, tag="lg")
                nc.gpsimd.dma_gather(
                    g_t[:], src_dram[:, :], idx_t[:],
                    num_idxs=n_in * 128, num_idxs_reg=n_in * 128, elem_size=D,
                    single_packet=False)
                gtiles[(si, grp)] = g_t
            return gtiles[(si, grp)][:, i % GCH, :]

        for i in range(nchunks):
            ps_lin = self.p_ps2.tile([128, Dout], f32, tag="lin")
            for si, (src, idx_dram) in enumerate(sources):
                if idx_dram is None:
                    ch = self.p_lin.tile([128, D], f32, tag="lch")
                    nc.sync.dma_start(ch[:], src[128 * i:128 * (i + 1), :])
                    src_view = ch[:]
                else:
                    src_view = gathered_view(si, i, src, idx_dram)
                ps_t = self.p_ps2.tile([128, D], f32, tag="tp")
                nc.tensor.transpose(ps_t[:], src_view, self.ident_t[:])
                tt = self.p_lin.tile([128, D], f32, tag="ltt")
                nc.vector.tensor_copy(tt[:], ps_t[:])
                nc.tensor.matmul(ps_lin[:], tt[:], Ws[si][:],
                                 start=(si == 0), stop=(si == len(sources) - 1))
            outt = self.p_lin.tile([128, Dout], f32, tag="lout")
            nc.vector.tensor_add(outt[:], ps_lin[:], bias_t[:])
            nc.sync.dma_start(dst_dram[128 * i:128 * (i + 1), :], outt[:])


# ---------------------------------------------------------------- main
def build(inputs, nphases=999):
    import concourse.bass as bass  # noqa: F401
    import concourse.tile as tile
    from concourse import bacc, mybir
    from contextlib import ExitStack

    X = np.ascontiguousarray(inputs["X"], np.float32)
    H = [
        (np.asarray(inputs["H0_v"]).astype(np.int64),
         np.asarray(inputs["H0_e"]).astype(np.int64), N0, E0),
        (np.asarray(inputs["H1_v"]).astype(np.int64),
         np.asarray(inputs["H1_e"]).astype(np.int64), N1, E1),
        (np.asarray(inputs["H2_v"]).astype(np.int64),
         np.asarray(inputs["H2_e"]).astype(np.int64), N2, E2),
    ]
    assign0 = np.asarray(inputs["assign0"]).astype(np.int64)
    assign1 = np.asarray(inputs["assign1"]).astype(np.int64)

    n0l, n1l, n2l = _pad_local(N0), _pad_local(N1), _pad_local(N2)
    e0p, e1p, e2p = _pad128(E0), _pad128(E1), _pad128(E2)

    def lap_streams(lv, nloc_pad):
        vi, ei, n, e = H[lv]
        dv_is, de_i = _degree_weights(vi, ei, n, e)
        owner, slot = vi % C, vi // C
        s1g, s1t, s1w, s2g, s2t, s2w = [], [], [], [], [], []
        for c in range(C):
            m = owner == c
            s1g.append(slot[m])
            s1t.append(ei[m])
            s1w.append(dv_is[vi[m]])
            s2g.append(ei[m])
            s2t.append(slot[m])
            s2w.append((dv_is[vi[m]] * de_i[ei[m]]).astype(np.float32))
        st1 = SegStage(f"l{lv}s1", s1g, s1t, s1w, _pad128(e))
        st2 = SegStage(f"l{lv}s2", s2g, s2t, s2w, nloc_pad)
        return st1, st2

    st1_0, st2_0 = lap_streams(0, n0l)
    st1_1, st2_1 = lap_streams(1, n1l)
    st1_2, st2_2 = lap_streams(2, n2l)

    def pool_streams(name, assign, nfine, ncoarse, ncl_pad):
        cnt = np.bincount(assign, minlength=ncoarse).astype(np.float32)
        inv = np.where(cnt > 0, 1.0 / cnt, 0.0).astype(np.float32)
        g = np.arange(nfine)
        owner, slot = g % C, g // C
        rows = (assign % C) * ncl_pad + assign // C
        gi, tr, wg = [], [], []
        for c in range(C):
            m = owner == c
            gi.append(slot[m])
            tr.append(rows[m])
            wg.append(inv[assign[m]])
        return SegStage(name, gi, tr, wg, C * ncl_pad)

    pool0 = pool_streams("pool0", assign0, N0, N1, n1l)
    pool1 = pool_streams("pool1", assign1, N1, N2, n2l)

    def unpool_planes(assign, nfine, ncl_pad):
        idxs = []
        for c in range(C):
            a = assign[np.arange(c, nfine, C)]
            idxs.append((a % C) * ncl_pad + a // C)
        return _gather_planes(idxs)

    up1_L, up1_planes = unpool_planes(assign1, N1, n2l)
    up0_L, up0_planes = unpool_planes(assign0, N0, n1l)

    nc = bacc.Bacc("TRN2", target_bir_lowering=False, debug=False,
                   num_devices=C)
    f32, i16 = mybir.dt.float32, mybir.dt.int16
    B = Builder(nc, mybir)

    x_arrs = []
    for c in range(C):
        xc = X[c::C]
        x_arrs.append(np.vstack([xc, np.zeros((n0l - len(xc), D_IN), np.float32)]))
    x_d = B.add_input("x", [n0l, D_IN], f32, x_arrs)
    out_d = nc.dram_tensor("out", [n0l, D_OUT], f32, kind="ExternalOutput")

    def dram(name, rows, d, shared=False):
        return nc.dram_tensor(name, [rows, d], f32,
                              addr_space="Shared" if shared else "Local")

    T0 = dram("T0", n0l, D_H)
    Y0p, Y0f = dram("Y0p", e0p, D_H), dram("Y0f", e0p, D_H, True)
    h0 = dram("h0", n0l, D_H)
    P1p, P1s = dram("P1p", C * n1l, D_H), dram("P1s", n1l, D_H)
    T1 = dram("T1", n1l, D_H)
    Y1p, Y1f = dram("Y1p", e1p, D_H), dram("Y1f", e1p, D_H, True)
    h1 = dram("h1", n1l, D_H)
    P2p, P2s = dram("P2p", C * n2l, D_H), dram("P2s", n2l, D_H)
    T2 = dram("T2", n2l, D_H)
    Y2p, Y2f = dram("Y2p", e2p, D_H), dram("Y2f", e2p, D_H, True)
    Xc2, Xc2f = dram("Xc2", n2l, D_H), dram("Xc2f", C * n2l, D_H, True)
    T3 = dram("T3", n1l, D_H)
    Y3p, Y3f = dram("Y3p", e1p, D_H), dram("Y3f", e1p, D_H, True)
    Xu1, Xuf = dram("Xu1", n1l, D_H), dram("Xuf", C * n1l, D_H, True)
    T4 = dram("T4", n0l, D_OUT)
    Y4p, Y4f = dram("Y4p", e0p, D_OUT), dram("Y4f", e0p, D_OUT, True)

    up1_d = B.add_input("up1_idx", [128, up1_L // 16], i16, up1_planes)
    up0_d = B.add_input("up0_idx", [128, up0_L // 16], i16, up0_planes)

    rg = [list(range(C))]

    def AR(src, dst):
        nc.gpsimd.collective_compute(
            "AllReduce", mybir.AluOpType.add, replica_groups=rg,
            ins=[src.ap().opt()], outs=[dst.ap().opt()])

    def RS(src, dst):
        nc.gpsimd.collective_compute(
            "ReduceScatter", mybir.AluOpType.add, replica_groups=rg,
            ins=[src.ap().opt()], outs=[dst.ap().opt()])

    def AG(src, dst):
        nc.gpsimd.collective_compute(
            "AllGather", mybir.AluOpType.bypass, replica_groups=rg,
            ins=[src.ap().opt()], outs=[dst.ap().opt()])

    with ExitStack() as ctx:
        tc = ctx.enter_context(tile.TileContext(nc))
        B.tc = tc
        B.setup_pools(ctx)
        B.setup_consts()
        W0t = B.const_mat("w0", inputs["W0"])
        W1t = B.const_mat("w1m", inputs["W1"])
        W2t = B.const_mat("w2m", inputs["W2"])
        W3a = B.const_mat("w3a", np.asarray(inputs["W3"])[:128])
        W3b = B.const_mat("w3b", np.asarray(inputs["W3"])[128:])
        W4a = B.const_mat("w4a", np.asarray(inputs["W4"])[:128])
        W4b = B.const_mat("w4b", np.asarray(inputs["W4"])[128:])
        b0t = B.const_mat("b0", np.tile(inputs["b0"], (128, 1)))
        b1t = B.const_mat("b1", np.tile(inputs["b1"], (128, 1)))
        b2t = B.const_mat("b2", np.tile(inputs["b2"], (128, 1)))
        b3t = B.const_mat("b3", np.tile(inputs["b3"], (128, 1)))
        b4t = B.const_mat("b4", np.tile(inputs["b4"], (128, 1)))

        phases = [
            lambda: B.emit_linear("lin0", [(x_d, None)], [W0t], b0t, T0, n0l // 128, D_H),
            lambda: B.emit_seg(st1_0, T0, Y0p, D_H, "copy"),
            lambda: AR(Y0p, Y0f),
            lambda: B.emit_seg(st2_0, Y0f, h0, D_H, "relu"),
            lambda: B.emit_seg(pool0, h0, P1p, D_H, "copy"),
            lambda: RS(P1p, P1s),
            lambda: B.emit_linear("lin1", [(P1s, None)], [W1t], b1t, T1, n1l // 128, D_H),
            lambda: B.emit_seg(st1_1, T1, Y1p, D_H, "copy"),
            lambda: AR(Y1p, Y1f),
            lambda: B.emit_seg(st2_1, Y1f, h1, D_H, "relu"),
            lambda: B.emit_seg(pool1, h1, P2p, D_H, "copy"),
            lambda: RS(P2p, P2s),
            lambda: B.emit_linear("lin2", [(P2s, None)], [W2t], b2t, T2, n2l // 128, D_H),
            lambda: B.emit_seg(st1_2, T2, Y2p, D_H, "copy"),
            lambda: AR(Y2p, Y2f),
            lambda: B.emit_seg(st2_2, Y2f, Xc2, D_H, "relu"),
            lambda: AG(Xc2, Xc2f),
            lambda: B.emit_linear("lin3", [(Xc2f, up1_d), (h1, None)], [W3a, W3b], b3t, T3, n1l // 128, D_H),
            lambda: B.emit_seg(st1_1.renamed("l1bs1"), T3, Y3p, D_H, "copy"),
            lambda: AR(Y3p, Y3f),
            lambda: B.emit_seg(st2_1.renamed("l1bs2"), Y3f, Xu1, D_H, "relu"),
            lambda: AG(Xu1, Xuf),
            lambda: B.emit_linear("lin4", [(Xuf, up0_d), (h0, None)], [W4a, W4b], b4t, T4, n0l // 128, D_OUT),
            lambda: B.emit_seg(st1_0.renamed("l0bs1"), T4, Y4p, D_OUT, "copy"),
            lambda: AR(Y4p, Y4f),
            lambda: B.emit_seg(st2_0.renamed("l0bs2"), Y4f, out_d, D_OUT, "copy"),
        ]
        for ph in phases[:nphases]:
            ph()
    nc.compile()

    in_maps = []
    for c in range(C):
        m = {}
        for name, arrs in B.inputs.items():
            m[name] = arrs[c] if isinstance(arrs, list) else arrs
        in_maps.append(m)
    return nc, in_maps


LAST_EXEC_NS = None


def _install_ntff_hook():
    import contextlib, ctypes, os, types
    try:
        from antenv import axon_hooks  # noqa: F401
        return
    except ImportError:
        pass
    import antenv
    so_path = os.environ.get("PJRT_LIBRARY_PATH", "/opt/axon/libaxon_pjrt.so")
    try:
        lib = ctypes.CDLL(so_path)
    except OSError:
        lib = None
    hook = None
    if lib is not None and hasattr(lib, "axon_start_nrt_profile"):
        lib.axon_start_nrt_profile.argtypes = [
            ctypes.POINTER(ctypes.c_int64), ctypes.c_size_t]
        lib.axon_start_nrt_profile.restype = ctypes.c_int64
        lib.axon_stop_nrt_profile.argtypes = [ctypes.c_char_p]
        lib.axon_stop_nrt_profile.restype = ctypes.c_int64

        @contextlib.contextmanager
        def hook(output_dir, device_ids):
            import jax
            jax.devices()
            if device_ids:
                ids = (ctypes.c_int64 * len(device_ids))(*device_ids)
                rc = lib.axon_start_nrt_profile(ids, len(device_ids))
            else:
                rc = lib.axon_start_nrt_profile(None, 0)
            if rc != 0:
                raise RuntimeError(f"axon_start_nrt_profile rc={rc}")
            try:
                yield
            finally:
                lib.axon_stop_nrt_profile(str(output_dir).encode())

    mod = types.ModuleType("antenv.axon_hooks")
    mod._hook = hook
    mod.get_axon_ntff_profile_hook = lambda: mod._hook
    def _set(h):
        mod._hook = h
    mod.set_axon_ntff_profile_hook = _set
    sys.modules["antenv.axon_hooks"] = mod
    antenv.axon_hooks = mod


def kernel(**inputs):
    global LAST_EXEC_NS
    import os
    trace = os.environ.get("HGNN_TRACE", "0") == "1"
    if trace:
        _install_ntff_hook()
    nc, in_maps = build(inputs)
    from concourse.bass_utils import run_bass_kernel_spmd
    res = run_bass_kernel_spmd(nc, in_maps, core_ids=list(range(C)),
                               trace=trace)
    LAST_EXEC_NS = res.exec_time_ns
    out = np.empty((N0, D_OUT), np.float32)
    for c in range(C):
        n = len(range(c, N0, C))
        out[c::C] = res.results[c]["out"][:n]
    return out



# revision 7
# speedup vs baseline: 1.4067x; 1.4067x over previous
"""Trainium2 Bass kernel for 3-level hierarchical hypergraph GNN (HGNN).

v2 design (8 NeuronCores, one SPMD NEFF, per-core index/one-hot data):
  - All feature tables fp16 (256B rows = dma_gather minimum element).
  - Segment-sum via identity-stationary matmuls: entries of each 128-row
    target window are layered so slot p of layer j holds the j-th entry
    targeting row p (pads gather a zero row). Overflow entries beyond the
    per-window layer count k_w go to host-precomputed 0/1 one-hot tail
    blocks (fp16, streamed from DRAM as lhsT).
  - All degree/pool normalizations folded into per-partition `scale` of
    the ACT-engine psum->SBUF flush (Relu fused where needed). The DVE is
    out of the hot path entirely (baseline bottleneck #1).
  - dma_gather descriptor generation (Q7, ~8ns/idx) spread over 4 SWDGE
    queues (num_swdge_queues=4, queue_num round-robin) -> 4 Q7 core pairs
    generate descriptors concurrently (baseline bottleneck #3: all on q0).
  - Linear layers: stationary = transposed input chunk (host-transposed X,
    HWDGE dma-transpose loads, or transpose-mode dma_gather for unpools),
    moving = weight; bias via rank-1 matmul; flush on ACT with dv scale.
  - Collectives in fp16 (half the bytes of the fp32 baseline).
"""
import sys

sys.path.insert(0, "/opt/trn_rl_repo")
import os
import numpy as np

C = 8
NQ = 4          # SWDGE queues to rotate over
CH = 24         # gather group size in 128-idx blocks
GCH = 16        # linear gather group in 128-row chunks
THETA = 0.55    # identity-layer utilization threshold
KMAX = 64

N0, N1, N2 = 100000, 25000, 6250
E0, E1, E2 = 20000, 5000, 1250
D_IN, D_H, D_OUT = 128, 128, 64


def _pad128(n):
    return ((n + 127) // 128) * 128


def _pad_local(n):
    return _pad128(-(-n // C))


# ---------------------------------------------------------------- host side
def _degrees(vi, ei, n, e):
    ones = np.ones(len(vi), np.float32)
    dV = np.bincount(vi, weights=ones, minlength=n)
    dE = np.bincount(ei, weights=ones, minlength=e)
    dv_is = np.where(dV > 0, dV ** -0.5, 0.0).astype(np.float32)
    de_i = np.where(dE > 0, 1.0 / dE, 0.0).astype(np.float32)
    return dv_is, de_i


def _plane_idx(idx):
    """int array (L,) -> [128, L//16] int16 (16-partition wrap, replicated x8)."""
    assert len(idx) % 16 == 0
    assert idx.max(initial=0) < 32768
    return np.tile(idx.astype(np.int16).reshape(-1, 16).T, (C, 1)).copy()


class SegStage:
    """Identity-layer + one-hot-tail schedule for one segment-sum stage.

    srcs/tgts: per-core int arrays; src = row in source table (local),
    tgt = row in destination table. n_src_pad = zero-row index in source.
    """

    def __init__(self, name, srcs, tgts, n_src_pad, n_tgt_pad, theta=THETA):
        self.name = name
        self.zrow = n_src_pad
        nw = n_tgt_pad // 128
        self.nw = nw

        # per-core per-target counts and entry occurrence ranks
        cnts = np.zeros((C, n_tgt_pad), np.int64)
        occs, orders = [], []
        for c in range(C):
            t = tgts[c]
            cnts[c] = np.bincount(t, minlength=n_tgt_pad)
            order = np.argsort(t, kind="stable")
            st = t[order]
            # occurrence rank within equal-target runs
            grp_start = np.r_[0, np.flatnonzero(np.diff(st)) + 1]
            run_len = np.diff(np.r_[grp_start, len(st)])
            occ_sorted = np.arange(len(st)) - np.repeat(grp_start, run_len)
            occ = np.empty(len(st), np.int64)
            occ[order] = occ_sorted
            occs.append(occ)
            orders.append(order)

        # pooled layer utilization per window -> k_w
        cap = np.minimum(cnts, KMAX)
        U = np.zeros((nw, KMAX + 1), np.int64)
        wid = np.arange(n_tgt_pad) // 128
        for c in range(C):
            np.add.at(U, (wid, cap[c]), 1)
        # ge[w, j] = #(core,target in w) with cnt >= j
        ge = U[:, ::-1].cumsum(axis=1)[:, ::-1]
        kws = np.zeros(nw, np.int64)
        for w in range(nw):
            k = 0
            while k < KMAX and ge[w, k + 1] >= theta * 128 * C:
                k += 1
            kws[w] = k
        self.kws = kws

        # tail block counts (uniform across cores)
        tails_per = np.zeros((C, nw), np.int64)
        for c in range(C):
            t = np.maximum(cnts[c] - kws[np.arange(n_tgt_pad) // 128], 0)
            tails_per[c] = np.bincount(wid, weights=t, minlength=nw).astype(np.int64)
        tbs = -(-tails_per.max(axis=0) // 128)
        # ensure windows with any entries produce blocks; empty windows get none
        has_any = (np.bincount(wid, weights=cnts.sum(axis=0), minlength=nw) > 0)
        self.tbs = np.where(has_any & (kws == 0) & (tbs == 0), 1, tbs)
        self.has_any = has_any

        # block list: per window, k_w identity blocks then tb_w one-hot blocks
        blocks = []  # (w, kind, layer_or_ohslot)
        oh_slot = 0
        for w in range(nw):
            if not has_any[w]:
                continue
            for j in range(kws[w]):
                blocks.append((w, 0, j))
            for i in range(self.tbs[w]):
                blocks.append((w, 1, oh_slot))
                oh_slot += 1
        self.blocks = blocks
        self.n_oh = oh_slot
        self.nblocks = len(blocks)
        self.L = 128 * self.nblocks

        # per-window block offset for vectorized fill
        blk_off = np.zeros(nw + 1, np.int64)
        for w in range(nw):
            blk_off[w + 1] = blk_off[w] + (kws[w] + self.tbs[w] if has_any[w] else 0)
        oh_off = np.zeros(nw, np.int64)
        s = 0
        for w in range(nw):
            oh_off[w] = s
            if has_any[w]:
                s += self.tbs[w]

        # build per-core idx streams and one-hot planes
        self.idx_arrs, self.oh_arrs = [], []
        for c in range(C):
            idx = np.full(self.L, self.zrow, np.int64)
            oh = np.zeros((max(self.n_oh, 1), 128, 128), np.float16)
            t = tgts[c]
            srcs_c = srcs[c]
            occ = occs[c]
            w_of = t // 128
            t_of = t % 128
            kw_of = kws[w_of]
            ident = occ < kw_of
            # identity entries: block = blk_off[w] + occ, slot = t_of
            b = blk_off[w_of[ident]] + occ[ident]
            idx[b * 128 + t_of[ident]] = srcs_c[ident]
            # tail entries: rank within (window) among tails, in stable order
            tm = ~ident
            tw = w_of[tm]
            order = np.argsort(tw, kind="stable")
            stw = tw[order]
            grp_start = np.r_[0, np.flatnonzero(np.diff(stw)) + 1]
            run_len = np.diff(np.r_[grp_start, len(stw)])
            rank_sorted = np.arange(len(stw)) - np.repeat(grp_start, run_len)
            rank = np.empty(len(stw), np.int64)
            rank[order] = rank_sorted
            tb = blk_off[tw] + kw_of[tm] + rank // 128
            ts = rank % 128
            idx[tb * 128 + ts] = srcs_c[tm]
            ohslot = oh_off[tw] + rank // 128
            oh[ohslot, ts, t_of[tm]] = 1.0
            self.idx_arrs.append(idx)
            self.oh_arrs.append(oh)

        self.idx_planes = [_plane_idx(a) for a in self.idx_arrs]
        if self.n_oh:
            # [128, n_oh*128]: block i cols [128i,128(i+1)), [slot p, target t]
            self.oh_planes = [
                np.ascontiguousarray(o.transpose(1, 0, 2).reshape(128, -1))
                for o in self.oh_arrs
            ]
        else:
            self.oh_planes = None

        # annotate first/last per window for psum start/stop
        self.first_last = []
        for i, (w, kind, j) in enumerate(blocks):
            first = i == 0 or blocks[i - 1][0] != w
            last = i == self.nblocks - 1 or blocks[i + 1][0] != w
            self.first_last.append((first, last))

    def renamed(self, name):
        st = SegStage.__new__(SegStage)
        st.__dict__ = dict(self.__dict__)
        st.name = name
        return st

    # ---- numpy emulation for self-test
    def emulate(self, src_tab, dst_rows, scale, relu, cols=128):
        """src_tab: per-core [n_src_pad+128, 128] f32. Returns per-core
        [nw*128, cols] f32 outputs (zeros for empty windows)."""
        outs = []
        for c in range(C):
            tab = src_tab[c]
            idx = self.idx_arrs[c]
            scflat = scale[c].T.reshape(-1)  # [128, nw] col-per-window -> flat
            out = np.zeros((self.nw * 128, cols), np.float32)
            ps = {}
            for i, (w, kind, j) in enumerate(self.blocks):
                g = tab[idx[i * 128:(i + 1) * 128]]
                if w not in ps:
                    ps[w] = np.zeros((128, 128), np.float32)
                if kind == 0:
                    ps[w] += g
                else:
                    ps[w] += self.oh_arrs[c][j].astype(np.float32).T @ g
                if self.first_last[i][1]:
                    r = ps.pop(w) * scflat[w * 128:(w + 1) * 128, None]
                    if relu:
                        r = np.maximum(r, 0.0)
                    out[w * 128:(w + 1) * 128] = r[:, :cols]
            outs.append(out)
        return outs


def _lin_gather_idx(idx_per_core, nchunks):
    """Pad per-core gather idx streams for linear sources to nchunks*128."""
    L = nchunks * 128
    planes = []
    arrs = []
    for c in range(C):
        gi = np.zeros(L, np.int64)
        gi[: len(idx_per_core[c])] = idx_per_core[c]
        arrs.append(gi)
        planes.append(_plane_idx(gi))
    return arrs, planes


# ---------------------------------------------------------------- device side
class Builder:
    def __init__(self, nc, mybir):
        self.nc = nc
        self.mybir = mybir
        self.tc = None
        self.inputs = {}
        self.qctr = 0

    def next_q(self):
        q = self.qctr % NQ
        self.qctr += 1
        return q

    def add_input(self, name, shape, dtype, arrays):
        assert name not in self.inputs, name
        t = self.nc.dram_tensor(name, list(shape), dtype, kind="ExternalInput")
        self.inputs[name] = arrays
        return t

    def setup_pools(self, ctx):
        tc = self.tc
        self.p_const = ctx.enter_context(tc.tile_pool(name="const", bufs=1))
        self.p_gath = ctx.enter_context(tc.tile_pool(name="gath", bufs=3))
        self.p_meta = ctx.enter_context(tc.tile_pool(name="meta", bufs=3))
        self.p_fl = ctx.enter_context(tc.tile_pool(name="fl", bufs=4))
        self.p_lin = ctx.enter_context(tc.tile_pool(name="lin", bufs=3))
        self.p_ps = ctx.enter_context(tc.tile_pool(name="ps", bufs=6, space="PSUM"))
        self.p_ps2 = ctx.enter_context(tc.tile_pool(name="ps2", bufs=2, space="PSUM"))

    def const_mat(self, name, arr, dtype=None):
        mybir = self.mybir
        a0 = arr[0] if isinstance(arr, list) else arr
        if dtype is None:
            dtype = mybir.dt.float32 if a0.dtype == np.float32 else mybir.dt.float16
        shape = list(a0.shape)
        d = self.add_input(name, shape, dtype, arr)
        t = self.p_const.tile(shape, dtype, tag=name)
        self.nc.sync.dma_start(t[:], d[:, :])
        return t

    def setup_consts(self):
        f16 = self.mybir.dt.float16
        self.ident_t = self.const_mat("c_ident", np.eye(128, dtype=np.float16))
        self.z16_t = self.const_mat("c_z16", np.zeros((128, 128), np.float16))
        self.z32_t = self.const_mat("c_z32", np.zeros((128, 128), np.float32))
        self.ones1_t = self.const_mat("c_ones1", np.ones((1, 128), np.float16))

    def emit_stage(self, st: SegStage, src_dram, dst_dram, scale_t, relu,
                   out_dtype=None, cols=128):
        nc, mybir = self.nc, self.mybir
        f32, f16, i16 = mybir.dt.float32, mybir.dt.float16, mybir.dt.int16
        if out_dtype is None:
            out_dtype = f16
        idx_d = self.add_input(f"{st.name}_idx", [128, st.L // 16], i16,
                               st.idx_planes)
        oh_d = None
        if st.n_oh:
            oh_d = self.add_input(f"{st.name}_oh", [128, st.n_oh * 128], f16,
                                  st.oh_planes)
        act = mybir.ActivationFunctionType
        func = act.Relu if relu else act.Copy

        ps = {}
        for start in range(0, st.nblocks, CH):
            nb = min(CH, st.nblocks - start)
            blks = st.blocks[start:start + nb]
            idx_t = self.p_meta.tile([128, nb * 8], i16, tag="idx")
            nc.sync.dma_start(idx_t[:], idx_d[:, start * 8:(start + nb) * 8])
            ohs = [j for (_, kind, j) in blks if kind == 1]
            oh_t, oh0 = None, 0
            if ohs:
                oh0 = ohs[0]
                noh = ohs[-1] - oh0 + 1
                oh_t = self.p_meta.tile([128, noh * 128], f16, tag="oh")
                nc.sync.dma_start(oh_t[:],
                                  oh_d[:, oh0 * 128:(oh0 + noh) * 128])
            g_t = self.p_gath.tile([128, nb, 128], f16, tag="g")
            nc.gpsimd.dma_gather(
                g_t[:], src_dram[:, :], idx_t[:],
                num_idxs=nb * 128, num_idxs_reg=nb * 128, elem_size=128,
                single_packet=False, queue_num=self.next_q())
            for k, (w, kind, j) in enumerate(blks):
                first, last = st.first_last[start + k]
                if first:
                    ps[w] = self.p_ps.tile([128, 128], f32, tag="seg", name="seg_ps")
                lhsT = self.ident_t[:] if kind == 0 else \
                    oh_t[:, (j - oh0) * 128:(j - oh0 + 1) * 128]
                nc.tensor.matmul(ps[w][:], lhsT, g_t[:, k, :],
                                 start=first, stop=last)
                if last:
                    fl = self.p_fl.tile([128, cols], out_dtype, tag=f"fl{cols}")
                    nc.scalar.activation(fl[:], ps[w][:, 0:cols], func,
                                         scale=scale_t[:, w:w + 1])
                    nc.sync.dma_start(dst_dram[128 * w:128 * (w + 1), 0:cols],
                                      fl[:])
                    del ps[w]
        # zero-fill windows with no entries
        ztile = self.z16_t if out_dtype == f16 else self.z32_t
        for w in range(st.nw):
            if not st.has_any[w]:
                nc.sync.dma_start(dst_dram[128 * w:128 * (w + 1), 0:cols],
                                  ztile[:, 0:cols])

    def emit_linear(self, name, sources, Ws, bias_t, scale_t, dst_dram,
                    nchunks):
        """dst chunk = (sum_s srcT_chunk_s.T @ Ws[s] + bias) * scale.

        sources: list of ('hostT', dram [128, n]) | ('dramT', dram [n, 128])
                 | ('gatherT', dram, idx_dram)."""
        nc, mybir = self.nc, self.mybir
        f32, f16, i16 = mybir.dt.float32, mybir.dt.float16, mybir.dt.int16
        act = mybir.ActivationFunctionType
        gtiles = {}

        def srcT(si, i, spec):
            kind = spec[0]
            if kind == "hostT":
                t = self.p_lin.tile([128, 128], f16, tag="lt")
                nc.sync.dma_start(t[:], spec[1][:, 128 * i:128 * (i + 1)])
                return t[:]
            if kind == "dramT":
                t = self.p_lin.tile([128, 128], f16, tag="lt")
                nc.sync.dma_start(t[:], spec[1][128 * i:128 * (i + 1), :],
                                  transpose=True)
                return t[:]
            grp = i // GCH
            if (si, grp) not in gtiles:
                n_in = min(GCH, nchunks - grp * GCH)
                idx_t = self.p_meta.tile([128, n_in * 8], i16, tag="lidx")
                nc.sync.dma_start(
                    idx_t[:],
                    spec[2][:, grp * GCH * 8:(grp * GCH + n_in) * 8])
                g_t = self.p_gath.tile([128, 1, n_in * 128], f16, tag="lg")
                nc.gpsimd.dma_gather(
                    g_t[:], spec[1][:, :], idx_t[:],
                    num_idxs=n_in * 128, num_idxs_reg=n_in * 128,
                    elem_size=128, transpose=True,
                    single_packet=False, queue_num=self.next_q())
                gtiles[(si, grp)] = g_t
            return gtiles[(si, grp)][:, 0, 128 * (i % GCH):128 * (i % GCH + 1)]

        for i in range(nchunks):
            ps = self.p_ps2.tile([128, 128], f32, tag="lin")
            for si, spec in enumerate(sources):
                nc.tensor.matmul(ps[:], srcT(si, i, spec), Ws[si][:],
                                 start=(si == 0), stop=False)
            nc.tensor.matmul(ps[:], self.ones1_t[:], bias_t[:],
                             start=False, stop=True)
            fl = self.p_fl.tile([128, 128], f16, tag="lfl")
            nc.scalar.activation(fl[:], ps[:], act.Copy,
                                 scale=scale_t[:, i:i + 1])
            nc.sync.dma_start(dst_dram[128 * i:128 * (i + 1), :], fl[:])


# ---------------------------------------------------------------- main
def prepare(inputs):
    """Host-side schedule construction (numpy only, no device imports)."""
    H = [
        (np.asarray(inputs["H0_v"]).astype(np.int64),
         np.asarray(inputs["H0_e"]).astype(np.int64), N0, E0),
        (np.asarray(inputs["H1_v"]).astype(np.int64),
         np.asarray(inputs["H1_e"]).astype(np.int64), N1, E1),
        (np.asarray(inputs["H2_v"]).astype(np.int64),
         np.asarray(inputs["H2_e"]).astype(np.int64), N2, E2),
    ]
    assign0 = np.asarray(inputs["assign0"]).astype(np.int64)
    assign1 = np.asarray(inputs["assign1"]).astype(np.int64)

    n0l, n1l, n2l = _pad_local(N0), _pad_local(N1), _pad_local(N2)
    e0p, e1p, e2p = _pad128(E0), _pad128(E1), _pad128(E2)
    P = {"n0l": n0l, "n1l": n1l, "n2l": n2l,
         "e0p": e0p, "e1p": e1p, "e2p": e2p}

    def lap(lv, nloc):
        vi, ei, n, e = H[lv]
        dv_is, de_i = _degrees(vi, ei, n, e)
        owner, slot = vi % C, vi // C
        s1s, s1t, s2s, s2t = [], [], [], []
        for c in range(C):
            m = owner == c
            s1s.append(slot[m]); s1t.append(ei[m])
            s2s.append(ei[m]); s2t.append(slot[m])
        ep = _pad128(e)
        st1 = SegStage(f"l{lv}s1", s1s, s1t, nloc, ep)
        st2 = SegStage(f"l{lv}s2", s2s, s2t, ep, nloc)
        # scale arrays: [128, nw] column-per-window (f32)
        de_pad = np.zeros(ep, np.float32); de_pad[:e] = de_i[:e]
        sc1 = np.ascontiguousarray(de_pad.reshape(-1, 128).T)
        sc2 = []
        for c in range(C):
            g = np.arange(c, n, C)
            dv_loc = np.zeros(nloc, np.float32)
            dv_loc[: len(g)] = dv_is[g]
            sc2.append(np.ascontiguousarray(dv_loc.reshape(-1, 128).T))
        return st1, st2, sc1, sc2

    st1_0, st2_0, de0_sc, dv0_sc = lap(0, n0l)
    st1_1, st2_1, de1_sc, dv1_sc = lap(1, n1l)
    st1_2, st2_2, de2_sc, dv2_sc = lap(2, n2l)

    def pool(name, assign, nfine, ncoarse, ncl):
        cnt = np.bincount(assign, minlength=ncoarse).astype(np.float32)
        inv = np.where(cnt > 0, 1.0 / cnt, 0.0).astype(np.float32)
        srcs, tgts = [], []
        for c in range(C):
            g = np.arange(c, nfine, C)
            a = assign[g]
            srcs.append(g // C)
            tgts.append((a % C) * ncl + a // C)
        nfl = _pad_local(nfine)
        st = SegStage(name, srcs, tgts, nfl, C * ncl)
        # inv over P rows: r -> cluster a = (r % ncl)*C + r//ncl
        rows = np.arange(C * ncl)
        a = (rows % ncl) * C + rows // ncl
        sc = np.where(a < ncoarse, inv[np.minimum(a, ncoarse - 1)], 0.0)
        sc = np.ascontiguousarray(sc.astype(np.float32).reshape(-1, 128).T)
        return st, sc

    pool0, inv1_sc = pool("pool0", assign0, N0, N1, n1l)
    pool1, inv2_sc = pool("pool1", assign1, N1, N2, n2l)

    def unpool_idx(assign, nfine, ncl, nchunks):
        idxs = []
        for c in range(C):
            a = assign[np.arange(c, nfine, C)]
            idxs.append((a % C) * ncl + a // C)
        return _lin_gather_idx(idxs, nchunks)

    up1_arrs, up1_planes = unpool_idx(assign1, N1, n2l, n1l // 128)
    up0_arrs, up0_planes = unpool_idx(assign0, N0, n1l, n0l // 128)

    # host-transposed per-core X shards (fp16)
    X = np.asarray(inputs["X"], np.float32)
    xT = []
    for c in range(C):
        xc = X[c::C].astype(np.float16)
        pad = np.zeros((n0l - len(xc), D_IN), np.float16)
        xT.append(np.ascontiguousarray(np.vstack([xc, pad]).T))

    W = {k: np.asarray(inputs[k], np.float32) for k in
         ("W0", "W1", "W2", "W3", "W4", "b0", "b1", "b2", "b3", "b4")}
    return dict(P=P, stages=dict(
        st1_0=st1_0, st2_0=st2_0, st1_1=st1_1, st2_1=st2_1,
        st1_2=st1_2, st2_2=st2_2, pool0=pool0, pool1=pool1),
        scales=dict(de0=de0_sc, dv0=dv0_sc, de1=de1_sc, dv1=dv1_sc,
                    de2=de2_sc, dv2=dv2_sc, inv1=inv1_sc, inv2=inv2_sc),
        up0=(up0_arrs, up0_planes), up1=(up1_arrs, up1_planes),
        xT=xT, W=W)


def emulate(prep, inputs):
    """Pure-numpy replay of the device schedule (fp32 math)."""
    P = prep["P"]; S = prep["stages"]; SC = prep["scales"]; W = prep["W"]
    n0l, n1l, n2l = P["n0l"], P["n1l"], P["n2l"]
    e0p, e1p, e2p = P["e0p"], P["e1p"], P["e2p"]

    def f16(a):
        return a.astype(np.float16).astype(np.float32)

    def lin(sources_T, Ws, b, scale_cols, rows):
        outs = []
        for c in range(C):
            acc = sum(sT.T @ w for sT, w in zip(sources_T[c], Ws)) + b
            sc = scale_cols[c] if isinstance(scale_cols, list) else scale_cols
            acc = acc * sc.T.reshape(-1)[:rows, None]
            outs.append(f16(acc))
        return outs

    def with_z(tabs, pad_rows=128):
        return [np.vstack([t, np.zeros((pad_rows, t.shape[1]), np.float32)])
                for t in tabs]

    xT = [t.astype(np.float32) for t in prep["xT"]]
    T0 = lin([[xT[c]] for c in range(C)], [f16(W["W0"])], W["b0"],
             SC["dv0"], n0l)
    Y0p = S["st1_0"].emulate(with_z(T0), e0p, [SC["de0"]] * C, False)
    Y0f = [f16(sum(Y0p))] * C
    h0 = S["st2_0"].emulate(with_z(Y0f), n0l, SC["dv0"], True)
    h0 = [f16(t) for t in h0]
    P1p = S["pool0"].emulate(with_z(h0), C * n1l, [SC["inv1"]] * C, False)
    P1s_full = sum(P1p)
    P1s = [f16(P1s_full[c * n1l:(c + 1) * n1l]) for c in range(C)]
    T1 = lin([[P1s[c].T] for c in range(C)], [f16(W["W1"])], W["b1"],
             SC["dv1"], n1l)
    Y1p = S["st1_1"].emulate(with_z(T1), e1p, [SC["de1"]] * C, False)
    Y1f = [f16(sum(Y1p))] * C
    h1 = S["st2_1"].emulate(with_z(Y1f), n1l, SC["dv1"], True)
    h1 = [f16(t) for t in h1]
    P2p = S["pool1"].emulate(with_z(h1), C * n2l, [SC["inv2"]] * C, False)
    P2s_full = sum(P2p)
    P2s = [f16(P2s_full[c * n2l:(c + 1) * n2l]) for c in range(C)]
    T2 = lin([[P2s[c].T] for c in range(C)], [f16(W["W2"])], W["b2"],
             SC["dv2"], n2l)
    Y2p = S["st1_2"].emulate(with_z(T2), e2p, [SC["de2"]] * C, False)
    Y2f = [f16(sum(Y2p))] * C
    Xc2 = S["st2_2"].emulate(with_z(Y2f), n2l, SC["dv2"], True)
    Xc2 = [f16(t) for t in Xc2]
    Xc2f = np.vstack(Xc2)  # [C*n2l, 128]
    Xc2fz = np.vstack([Xc2f, np.zeros((128, 128), np.float32)])
    up0_arrs, _ = prep["up0"]; up1_arrs, _ = prep["up1"]
    W3 = f16(W["W3"]); W4 = f16(W["W4"])
    T3 = lin([[Xc2fz[up1_arrs[c]].T, h1[c].T] for c in range(C)],
             [W3[:128], W3[128:]], W["b3"], SC["dv1"], n1l)
    Y3p = S["st1_1"].emulate(with_z(T3), e1p, [SC["de1"]] * C, False)
    Y3f = [f16(sum(Y3p))] * C
    Xu1 = S["st2_1"].emulate(with_z(Y3f), n1l, SC["dv1"], True)
    Xu1 = [f16(t) for t in Xu1]
    Xuf = np.vstack(Xu1)
    Xufz = np.vstack([Xuf, np.zeros((128, 128), np.float32)])
    W4p = np.zeros((256, 128), np.float32); W4p[:, :64] = W4
    b4p = np.zeros(128, np.float32); b4p[:64] = W["b4"]
    T4 = lin([[Xufz[up0_arrs[c]].T, h0[c].T] for c in range(C)],
             [W4p[:128], W4p[128:]], b4p, SC["dv0"], n0l)
    Y4p = S["st1_0"].emulate(with_z(T4), e0p, [SC["de0"]] * C, False)
    Y4f = [f16(sum(Y4p))] * C
    outs = S["st2_0"].emulate(with_z(Y4f), n0l, SC["dv0"], False, cols=64)
    out = np.empty((N0, D_OUT), np.float32)
    for c in range(C):
        n = len(range(c, N0, C))
        out[c::C] = outs[c][:n]
    return out


def build(prep):
    import concourse.bass as bass  # noqa: F401
    import concourse.tile as tile
    from concourse import bacc, mybir
    from contextlib import ExitStack

    P = prep["P"]; S = prep["stages"]; SC = prep["scales"]; W = prep["W"]
    n0l, n1l, n2l = P["n0l"], P["n1l"], P["n2l"]
    e0p, e1p, e2p = P["e0p"], P["e1p"], P["e2p"]

    nc = bacc.Bacc("TRN2", target_bir_lowering=False, debug=False,
                   num_devices=C, num_swdge_queues=NQ)
    f32, f16, i16 = mybir.dt.float32, mybir.dt.float16, mybir.dt.int16
    B = Builder(nc, mybir)

    def dram(name, rows, d=128, dt=None, shared=False):
        return nc.dram_tensor(name, [rows, d], dt or f16,
                              addr_space="Shared" if shared else "Local")

    # tables (+128 zero window on every gather source)
    T0 = dram("T0", n0l + 128)
    Y0p = dram("Y0p", e0p); Y0f = dram("Y0f", e0p + 128, shared=True)
    h0 = dram("h0", n0l + 128)
    P1p = dram("P1p", C * n1l); P1s = dram("P1s", n1l)
    T1 = dram("T1", n1l + 128)
    Y1p = dram("Y1p", e1p); Y1f = dram("Y1f", e1p + 128, shared=True)
    h1 = dram("h1", n1l + 128)
    P2p = dram("P2p", C * n2l); P2s = dram("P2s", n2l)
    T2 = dram("T2", n2l + 128)
    Y2p = dram("Y2p", e2p); Y2f = dram("Y2f", e2p + 128, shared=True)
    Xc2 = dram("Xc2", n2l); Xc2f = dram("Xc2f", C * n2l + 128, shared=True)
    T3 = dram("T3", n1l + 128)
    Y3p = dram("Y3p", e1p); Y3f = dram("Y3f", e1p + 128, shared=True)
    Xu1 = dram("Xu1", n1l); Xuf = dram("Xuf", C * n1l + 128, shared=True)
    T4 = dram("T4", n0l + 128)
    Y4p = dram("Y4p", e0p); Y4f = dram("Y4f", e0p + 128, shared=True)
    out_d = nc.dram_tensor("out", [n0l, D_OUT], f32, kind="ExternalOutput")

    xT_d = B.add_input("xT", [128, n0l], f16, prep["xT"])
    up0_d = B.add_input("up0_idx", list(prep["up0"][1][0].shape), i16,
                        prep["up0"][1])
    up1_d = B.add_input("up1_idx", list(prep["up1"][1][0].shape), i16,
                        prep["up1"][1])

    rg = [list(range(C))]

    def coll(kind, src_ap, dst_ap):
        op = mybir.AluOpType.bypass if kind == "AllGather" else \
            mybir.AluOpType.add
        nc.gpsimd.collective_compute(
            kind, op, replica_groups=rg,
            ins=[src_ap.opt()], outs=[dst_ap.opt()])

    with ExitStack() as ctx:
        tc = ctx.enter_context(tile.TileContext(nc))
        B.tc = tc
        B.setup_pools(ctx)
        B.setup_consts()
        W0t = B.const_mat("w0", W["W0"].astype(np.float16))
        W1t = B.const_mat("w1", W["W1"].astype(np.float16))
        W2t = B.const_mat("w2", W["W2"].astype(np.float16))
        W3a = B.const_mat("w3a", W["W3"][:128].astype(np.float16))
        W3b = B.const_mat("w3b", W["W3"][128:].astype(np.float16))
        W4pad = np.zeros((256, 128), np.float16)
        W4pad[:, :64] = W["W4"].astype(np.float16)
        W4a = B.const_mat("w4a", W4pad[:128])
        W4b = B.const_mat("w4b", W4pad[128:])
        b4pad = np.zeros(128, np.float32); b4pad[:64] = W["b4"]
        bts = {}
        for k, v in (("b0", W["b0"]), ("b1", W["b1"]), ("b2", W["b2"]),
                     ("b3", W["b3"]), ("b4", b4pad)):
            bts[k] = B.const_mat(k, v.astype(np.float16).reshape(1, 128))
        scs = {}
        for k in ("de0", "de1", "de2", "inv1", "inv2"):
            scs[k] = B.const_mat("sc_" + k, SC[k])
        for k in ("dv0", "dv1", "dv2"):
            scs[k] = B.const_mat("sc_" + k, SC[k])

        # zero windows for all gather-source tables
        for t, rows in ((T0, n0l), (T1, n1l), (T2, n2l), (T3, n1l),
                        (T4, n0l), (Y0f, e0p), (Y1f, e1p), (Y2f, e2p),
                        (Y3f, e1p), (Y4f, e0p), (h0, n0l), (h1, n1l),
                        (Xc2f, C * n2l), (Xuf, C * n1l)):
            nc.sync.dma_start(t[rows:rows + 128, :], B.z16_t[:])

        B.emit_linear("lin0", [("hostT", xT_d)], [W0t], bts["b0"],
                      scs["dv0"], T0, n0l // 128)
        B.emit_stage(S["st1_0"], T0, Y0p, scs["de0"], False)
        coll("AllReduce", Y0p.ap(), Y0f[0:e0p, :])
        B.emit_stage(S["st2_0"], Y0f, h0, scs["dv0"], True)
        B.emit_stage(S["pool0"], h0, P1p, scs["inv1"], False)
        coll("ReduceScatter", P1p.ap(), P1s.ap())
        B.emit_linear("lin1", [("dramT", P1s)], [W1t], bts["b1"],
                      scs["dv1"], T1, n1l // 128)
        B.emit_stage(S["st1_1"], T1, Y1p, scs["de1"], False)
        coll("AllReduce", Y1p.ap(), Y1f[0:e1p, :])
        B.emit_stage(S["st2_1"], Y1f, h1, scs["dv1"], True)
        B.emit_stage(S["pool1"], h1, P2p, scs["inv2"], False)
        coll("ReduceScatter", P2p.ap(), P2s.ap())
        B.emit_linear("lin2", [("dramT", P2s)], [W2t], bts["b2"],
                      scs["dv2"], T2, n2l // 128)
        B.emit_stage(S["st1_2"], T2, Y2p, scs["de2"], False)
        coll("AllReduce", Y2p.ap(), Y2f[0:e2p, :])
        B.emit_stage(S["st2_2"], Y2f, Xc2, scs["dv2"], True)
        coll("AllGather", Xc2.ap(), Xc2f[0:C * n2l, :])
        B.emit_linear("lin3", [("gatherT", Xc2f, up1_d), ("dramT", h1)],
                      [W3a, W3b], bts["b3"], scs["dv1"], T3, n1l // 128)
        B.emit_stage(S["st1_1"].renamed("l1bs1"), T3, Y3p, scs["de1"], False)
        coll("AllReduce", Y3p.ap(), Y3f[0:e1p, :])
        B.emit_stage(S["st2_1"].renamed("l1bs2"), Y3f, Xu1, scs["dv1"], True)
        coll("AllGather", Xu1.ap(), Xuf[0:C * n1l, :])
        B.emit_linear("lin4", [("gatherT", Xuf, up0_d), ("dramT", h0)],
                      [W4a, W4b], bts["b4"], scs["dv0"], T4, n0l // 128)
        B.emit_stage(S["st1_0"].renamed("l0bs1"), T4, Y4p, scs["de0"], False)
        coll("AllReduce", Y4p.ap(), Y4f[0:e0p, :])
        B.emit_stage(S["st2_0"].renamed("l0bs2"), Y4f, out_d, scs["dv0"],
                     False, out_dtype=f32, cols=D_OUT)
    nc.compile()

    in_maps = []
    for c in range(C):
        m = {}
        for name, arrs in B.inputs.items():
            m[name] = arrs[c] if isinstance(arrs, list) else arrs
        in_maps.append(m)
    return nc, in_maps


LAST_EXEC_NS = None


def _install_ntff_hook():
    import contextlib, ctypes, types
    try:
        from antenv import axon_hooks  # noqa: F401
        return
    except ImportError:
        pass
    import antenv
    so_path = os.environ.get("PJRT_LIBRARY_PATH", "/opt/axon/libaxon_pjrt.so")
    try:
        lib = ctypes.CDLL(so_path)
    except OSError:
        lib = None
    hook = None
    if lib is not None and hasattr(lib, "axon_start_nrt_profile"):
        lib.axon_start_nrt_profile.argtypes = [
            ctypes.POINTER(ctypes.c_int64), ctypes.c_size_t]
        lib.axon_start_nrt_profile.restype = ctypes.c_int64
        lib.axon_stop_nrt_profile.argtypes = [ctypes.c_char_p]
        lib.axon_stop_nrt_profile.restype = ctypes.c_int64

        @contextlib.contextmanager
        def hook(output_dir, device_ids):
            import jax
            jax.devices()
            if device_ids:
                ids = (ctypes.c_int64 * len(device_ids))(*device_ids)
                rc = lib.axon_start_nrt_profile(ids, len(device_ids))
            else:
                rc = lib.axon_start_nrt_profile(None, 0)
            if rc != 0:
                raise RuntimeError(f"axon_start_nrt_profile rc={rc}")
            try:
                yield
            finally:
                lib.axon_stop_nrt_profile(str(output_dir).encode())

    mod = types.ModuleType("antenv.axon_hooks")
    mod._hook = hook
    mod.get_axon_ntff_profile_hook = lambda: mod._hook

    def _set(h):
        mod._hook = h
    mod.set_axon_ntff_profile_hook = _set
    sys.modules["antenv.axon_hooks"] = mod
    antenv.axon_hooks = mod


def kernel(**inputs):
    global LAST_EXEC_NS
    prep = prepare(inputs)
    if os.environ.get("HGNN_EMULATE", "0") == "1":
        return emulate(prep, inputs)
    trace = os.environ.get("HGNN_TRACE", "0") == "1"
    if trace:
        _install_ntff_hook()
    nc, in_maps = build(prep)
    from concourse.bass_utils import run_bass_kernel_spmd
    res = run_bass_kernel_spmd(nc, in_maps, core_ids=list(range(C)),
                               trace=trace)
    LAST_EXEC_NS = res.exec_time_ns
    out = np.empty((N0, D_OUT), np.float32)
    for c in range(C):
        n = len(range(c, N0, C))
        out[c::C] = res.results[c]["out"][:n]
    return out


# revision 15
# speedup vs baseline: 1.5417x; 1.0960x over previous
"""Trainium2 Bass kernel for 3-level hierarchical hypergraph GNN (HGNN).

v2 design (8 NeuronCores, one SPMD NEFF, per-core index/one-hot data):
  - All feature tables fp16 (256B rows = dma_gather minimum element).
  - Segment-sum via identity-stationary matmuls: entries of each 128-row
    target window are layered so slot p of layer j holds the j-th entry
    targeting row p (pads gather a zero row). Overflow entries beyond the
    per-window layer count k_w go to host-precomputed 0/1 one-hot tail
    blocks (fp16, streamed from DRAM as lhsT).
  - All degree/pool normalizations folded into per-partition `scale` of
    the ACT-engine psum->SBUF flush (Relu fused where needed). The DVE is
    out of the hot path entirely (baseline bottleneck #1).
  - dma_gather descriptor generation (Q7, ~8ns/idx) spread over 4 SWDGE
    queues (num_swdge_queues=4, queue_num round-robin) -> 4 Q7 core pairs
    generate descriptors concurrently (baseline bottleneck #3: all on q0).
  - Linear layers: stationary = transposed input chunk (host-transposed X,
    HWDGE dma-transpose loads, or transpose-mode dma_gather for unpools),
    moving = weight; bias via rank-1 matmul; flush on ACT with dv scale.
  - Collectives in fp16 (half the bytes of the fp32 baseline).
"""
import sys

sys.path.insert(0, "/opt/trn_rl_repo")
import os
import numpy as np

C = 8
NQ = 4          # SWDGE queues to rotate over
CH = 24         # gather group size in 128-idx blocks
GCH = 16        # linear gather group in 128-row chunks
THETA = 0.55    # identity-layer utilization threshold
KMAX = 64

N0, N1, N2 = 100000, 25000, 6250
E0, E1, E2 = 20000, 5000, 1250
D_IN, D_H, D_OUT = 128, 128, 64


def _pad128(n):
    return ((n + 127) // 128) * 128


def _pad_local(n):
    return _pad128(-(-n // C))


# ---------------------------------------------------------------- host side
def _degrees(vi, ei, n, e):
    ones = np.ones(len(vi), np.float32)
    dV = np.bincount(vi, weights=ones, minlength=n)
    dE = np.bincount(ei, weights=ones, minlength=e)
    dv_is = np.where(dV > 0, dV ** -0.5, 0.0).astype(np.float32)
    de_i = np.where(dE > 0, 1.0 / dE, 0.0).astype(np.float32)
    return dv_is, de_i


def _plane_idx(idx):
    """int array (L,) -> [128, L//16] int16 (16-partition wrap, replicated x8)."""
    assert len(idx) % 16 == 0
    assert idx.max(initial=0) < 32768
    return np.tile(idx.astype(np.int16).reshape(-1, 16).T, (C, 1)).copy()


class SegStage:
    """Identity-layer + one-hot-tail schedule for one segment-sum stage.

    srcs/tgts: per-core int arrays; src = row in source table (local),
    tgt = row in destination table. n_src_pad = zero-row index in source.
    """

    def __init__(self, name, srcs, tgts, n_src_pad, n_tgt_pad, theta=THETA):
        self.name = name
        self.zrow = n_src_pad
        nw = n_tgt_pad // 128
        self.nw = nw

        # per-core per-target counts and entry occurrence ranks
        cnts = np.zeros((C, n_tgt_pad), np.int64)
        occs, orders = [], []
        for c in range(C):
            t = tgts[c]
            cnts[c] = np.bincount(t, minlength=n_tgt_pad)
            order = np.argsort(t, kind="stable")
            st = t[order]
            # occurrence rank within equal-target runs
            grp_start = np.r_[0, np.flatnonzero(np.diff(st)) + 1]
            run_len = np.diff(np.r_[grp_start, len(st)])
            occ_sorted = np.arange(len(st)) - np.repeat(grp_start, run_len)
            occ = np.empty(len(st), np.int64)
            occ[order] = occ_sorted
            occs.append(occ)
            orders.append(order)

        # pooled layer utilization per window -> k_w
        cap = np.minimum(cnts, KMAX)
        U = np.zeros((nw, KMAX + 1), np.int64)
        wid = np.arange(n_tgt_pad) // 128
        for c in range(C):
            np.add.at(U, (wid, cap[c]), 1)
        # ge[w, j] = #(core,target in w) with cnt >= j
        ge = U[:, ::-1].cumsum(axis=1)[:, ::-1]
        kws = np.zeros(nw, np.int64)
        for w in range(nw):
            k = 0
            while k < KMAX and ge[w, k + 1] >= theta * 128 * C:
                k += 1
            kws[w] = k
        self.kws = kws

        # tail block counts (uniform across cores)
        tails_per = np.zeros((C, nw), np.int64)
        for c in range(C):
            t = np.maximum(cnts[c] - kws[np.arange(n_tgt_pad) // 128], 0)
            tails_per[c] = np.bincount(wid, weights=t, minlength=nw).astype(np.int64)
        tbs = -(-tails_per.max(axis=0) // 128)
        # ensure windows with any entries produce blocks; empty windows get none
        has_any = (np.bincount(wid, weights=cnts.sum(axis=0), minlength=nw) > 0)
        self.tbs = np.where(has_any & (kws == 0) & (tbs == 0), 1, tbs)
        self.has_any = has_any

        # block list: per window, k_w identity blocks then tb_w one-hot blocks
        blocks = []  # (w, kind, layer_or_ohslot)
        oh_slot = 0
        for w in range(nw):
            if not has_any[w]:
                continue
            for j in range(kws[w]):
                blocks.append((w, 0, j))
            for i in range(self.tbs[w]):
                blocks.append((w, 1, oh_slot))
                oh_slot += 1
        self.blocks = blocks
        self.n_oh = oh_slot
        self.nblocks = len(blocks)
        self.L = 128 * self.nblocks

        # per-window block offset for vectorized fill
        blk_off = np.zeros(nw + 1, np.int64)
        for w in range(nw):
            blk_off[w + 1] = blk_off[w] + (kws[w] + self.tbs[w] if has_any[w] else 0)
        oh_off = np.zeros(nw, np.int64)
        s = 0
        for w in range(nw):
            oh_off[w] = s
            if has_any[w]:
                s += self.tbs[w]

        # build per-core idx streams and one-hot planes
        self.idx_arrs, self.oh_arrs = [], []
        for c in range(C):
            idx = np.full(self.L, self.zrow, np.int64)
            oh = np.zeros((max(self.n_oh, 1), 128, 128), np.float16)
            t = tgts[c]
            srcs_c = srcs[c]
            occ = occs[c]
            w_of = t // 128
            t_of = t % 128
            kw_of = kws[w_of]
            ident = occ < kw_of
            # identity entries: block = blk_off[w] + occ, slot = t_of
            b = blk_off[w_of[ident]] + occ[ident]
            idx[b * 128 + t_of[ident]] = srcs_c[ident]
            # tail entries: rank within (window) among tails, in stable order
            tm = ~ident
            tw = w_of[tm]
            order = np.argsort(tw, kind="stable")
            stw = tw[order]
            grp_start = np.r_[0, np.flatnonzero(np.diff(stw)) + 1]
            run_len = np.diff(np.r_[grp_start, len(stw)])
            rank_sorted = np.arange(len(stw)) - np.repeat(grp_start, run_len)
            rank = np.empty(len(stw), np.int64)
            rank[order] = rank_sorted
            tb = blk_off[tw] + kw_of[tm] + rank // 128
            ts = rank % 128
            idx[tb * 128 + ts] = srcs_c[tm]
            ohslot = oh_off[tw] + rank // 128
            oh[ohslot, ts, t_of[tm]] = 1.0
            self.idx_arrs.append(idx)
            self.oh_arrs.append(oh)

        self.idx_planes = [_plane_idx(a) for a in self.idx_arrs]
        if self.n_oh:
            # [128, n_oh*128]: block i cols [128i,128(i+1)), [slot p, target t]
            self.oh_planes = [
                np.ascontiguousarray(o.transpose(1, 0, 2).reshape(128, -1))
                for o in self.oh_arrs
            ]
        else:
            self.oh_planes = None

        # annotate first/last per window for psum start/stop
        self.first_last = []
        for i, (w, kind, j) in enumerate(blocks):
            first = i == 0 or blocks[i - 1][0] != w
            last = i == self.nblocks - 1 or blocks[i + 1][0] != w
            self.first_last.append((first, last))

    def renamed(self, name):
        st = SegStage.__new__(SegStage)
        st.__dict__ = dict(self.__dict__)
        st.name = name
        return st

    # ---- numpy emulation for self-test
    def emulate(self, src_tab, dst_rows, scale, relu, cols=128):
        """src_tab: per-core [n_src_pad+128, 128] f32. Returns per-core
        [nw*128, cols] f32 outputs (zeros for empty windows)."""
        outs = []
        for c in range(C):
            tab = src_tab[c]
            idx = self.idx_arrs[c]
            scflat = scale[c].T.reshape(-1)  # [128, nw] col-per-window -> flat
            out = np.zeros((self.nw * 128, cols), np.float32)
            ps = {}
            for i, (w, kind, j) in enumerate(self.blocks):
                g = tab[idx[i * 128:(i + 1) * 128]]
                if w not in ps:
                    ps[w] = np.zeros((128, 128), np.float32)
                if kind == 0:
                    ps[w] += g
                else:
                    ps[w] += self.oh_arrs[c][j].astype(np.float32).T @ g
                if self.first_last[i][1]:
                    r = ps.pop(w) * scflat[w * 128:(w + 1) * 128, None]
                    if relu:
                        r = np.maximum(r, 0.0)
                    out[w * 128:(w + 1) * 128] = r[:, :cols]
            outs.append(out)
        return outs


def _lin_gather_idx(idx_per_core, nchunks):
    """Pad per-core gather idx streams for linear sources to nchunks*128."""
    L = nchunks * 128
    planes = []
    arrs = []
    for c in range(C):
        gi = np.zeros(L, np.int64)
        gi[: len(idx_per_core[c])] = idx_per_core[c]
        arrs.append(gi)
        planes.append(_plane_idx(gi))
    return arrs, planes


# ---------------------------------------------------------------- device side
class Builder:
    def __init__(self, nc, mybir):
        self.nc = nc
        self.mybir = mybir
        self.tc = None
        self.inputs = {}
        self.qctr = 0

    def next_q(self):
        q = self.qctr % NQ
        self.qctr += 1
        return q

    def add_input(self, name, shape, dtype, arrays):
        assert name not in self.inputs, name
        t = self.nc.dram_tensor(name, list(shape), dtype, kind="ExternalInput")
        self.inputs[name] = arrays
        return t

    def setup_pools(self, ctx):
        tc = self.tc
        self.p_const = ctx.enter_context(tc.tile_pool(name="const", bufs=1))
        self.p_gath = ctx.enter_context(tc.tile_pool(name="gath", bufs=3))
        self.p_meta = ctx.enter_context(tc.tile_pool(name="meta", bufs=3))
        self.p_fl = ctx.enter_context(tc.tile_pool(name="fl", bufs=4))
        self.p_lin = ctx.enter_context(tc.tile_pool(name="lin", bufs=3))
        self.p_ps = ctx.enter_context(tc.tile_pool(name="ps", bufs=4, space="PSUM"))
        self.p_ps2 = ctx.enter_context(tc.tile_pool(name="ps2", bufs=2, space="PSUM"))

    def const_mat(self, name, arr, dtype=None):
        mybir = self.mybir
        a0 = arr[0] if isinstance(arr, list) else arr
        if dtype is None:
            dtype = mybir.dt.float32 if a0.dtype == np.float32 else mybir.dt.float16
        shape = list(a0.shape)
        d = self.add_input(name, shape, dtype, arr)
        t = self.p_const.tile(shape, dtype, tag=name)
        self.nc.sync.dma_start(t[:], d[:, :])
        return t

    def setup_consts(self):
        f16 = self.mybir.dt.float16
        self.ident_t = self.const_mat("c_ident", np.eye(128, dtype=np.float16))
        self.z16_t = self.const_mat("c_z16", np.zeros((128, 128), np.float16))
        self.z32_t = self.const_mat("c_z32", np.zeros((128, 128), np.float32))
        self.ones1_t = self.const_mat("c_ones1", np.ones((1, 128), np.float16))

    def emit_stage(self, st: SegStage, src_dram, dst_dram, scale_t, relu,
                   out_dtype=None, cols=128):
        nc, mybir = self.nc, self.mybir
        f32, f16, i16 = mybir.dt.float32, mybir.dt.float16, mybir.dt.int16
        if out_dtype is None:
            out_dtype = f16
        idx_d = self.add_input(f"{st.name}_idx", [128, st.L // 16], i16,
                               st.idx_planes)
        oh_d = None
        if st.n_oh:
            oh_d = self.add_input(f"{st.name}_oh", [128, st.n_oh * 128], f16,
                                  st.oh_planes)
        act = mybir.ActivationFunctionType
        func = act.Relu if relu else act.Copy

        ps = {}
        for start in range(0, st.nblocks, CH):
            nb = min(CH, st.nblocks - start)
            blks = st.blocks[start:start + nb]
            idx_t = self.p_meta.tile([128, nb * 8], i16, tag="idx")
            nc.sync.dma_start(idx_t[:], idx_d[:, start * 8:(start + nb) * 8])
            ohs = [j for (_, kind, j) in blks if kind == 1]
            oh_t, oh0 = None, 0
            if ohs:
                oh0 = ohs[0]
                noh = ohs[-1] - oh0 + 1
                oh_t = self.p_meta.tile([128, noh * 128], f16, tag="oh")
                nc.sync.dma_start(oh_t[:],
                                  oh_d[:, oh0 * 128:(oh0 + noh) * 128])
            g_t = self.p_gath.tile([128, nb, 128], f16, tag="g")
            nc.gpsimd.dma_gather(
                g_t[:], src_dram[:, :], idx_t[:],
                num_idxs=nb * 128, num_idxs_reg=nb * 128, elem_size=128,
                single_packet=False, queue_num=self.next_q())
            for k, (w, kind, j) in enumerate(blks):
                first, last = st.first_last[start + k]
                if first:
                    ps[w] = self.p_ps.tile([128, 128], f32, tag="seg", name="seg_ps")
                lhsT = self.ident_t[:] if kind == 0 else \
                    oh_t[:, (j - oh0) * 128:(j - oh0 + 1) * 128]
                nc.tensor.matmul(ps[w][:], lhsT, g_t[:, k, :],
                                 start=first, stop=last)
                if last:
                    fl = self.p_fl.tile([128, cols], out_dtype, tag=f"fl{cols}")
                    nc.scalar.activation(fl[:], ps[w][:, 0:cols], func,
                                         scale=scale_t[:, w:w + 1])
                    nc.sync.dma_start(dst_dram[128 * w:128 * (w + 1), 0:cols],
                                      fl[:])
                    del ps[w]
        # zero-fill windows with no entries
        ztile = self.z16_t if out_dtype == f16 else self.z32_t
        for w in range(st.nw):
            if not st.has_any[w]:
                nc.sync.dma_start(dst_dram[128 * w:128 * (w + 1), 0:cols],
                                  ztile[:, 0:cols])

    def emit_fused_stage(self, st: SegStage, xexpT_d, dvb_d, W_t, brow_t,
                         scale_t, dst_dram):
        """st1_0 with lin0 fused: rhs tiles computed inline from host-expanded
        transposed input (slab loads, no gather)."""
        nc, mybir = self.nc, self.mybir
        f32, f16 = mybir.dt.float32, mybir.dt.float16
        act = mybir.ActivationFunctionType
        oh_d = None
        if st.n_oh:
            oh_d = self.add_input(f"{st.name}_oh", [128, st.n_oh * 128], f16,
                                  st.oh_planes)
        ps = {}
        for start in range(0, st.nblocks, CH):
            nb = min(CH, st.nblocks - start)
            blks = st.blocks[start:start + nb]
            slab = self.p_gath.tile([128, nb * 128], f16, tag="xslab")
            nc.sync.dma_start(slab[:],
                              xexpT_d[:, start * 128:(start + nb) * 128])
            dvb_t = self.p_meta.tile([1, nb * 128], f16, tag="dvb")
            nc.sync.dma_start(dvb_t[:],
                              dvb_d[:, start * 128:(start + nb) * 128])
            ohs = [j for (_, kind, j) in blks if kind == 1]
            oh_t, oh0 = None, 0
            if ohs:
                oh0 = ohs[0]
                noh = ohs[-1] - oh0 + 1
                oh_t = self.p_meta.tile([128, noh * 128], f16, tag="oh")
                nc.sync.dma_start(oh_t[:],
                                  oh_d[:, oh0 * 128:(oh0 + noh) * 128])
            for k, (w, kind, j) in enumerate(blks):
                first, last = st.first_last[start + k]
                ps_lin = self.p_ps2.tile([128, 128], f32, tag="flin",
                                         name="flin_ps")
                nc.tensor.matmul(ps_lin[:], slab[:, k * 128:(k + 1) * 128],
                                 W_t[:], start=True, stop=False)
                nc.tensor.matmul(ps_lin[:], dvb_t[:, k * 128:(k + 1) * 128],
                                 brow_t[:], start=False, stop=True)
                fl_lin = self.p_lin.tile([128, 128], f16, tag="flin_s")
                nc.vector.tensor_copy(fl_lin[:], ps_lin[:])
                if first:
                    ps[w] = self.p_ps.tile([128, 128], f32, tag="seg",
                                           name="seg_ps")
                lhsT = self.ident_t[:] if kind == 0 else \
                    oh_t[:, (j - oh0) * 128:(j - oh0 + 1) * 128]
                nc.tensor.matmul(ps[w][:], lhsT, fl_lin[:],
                                 start=first, stop=last)
                if last:
                    fl = self.p_fl.tile([128, 128], f16, tag="fl128")
                    nc.scalar.activation(fl[:], ps[w][:], act.Copy,
                                         scale=scale_t[:, w:w + 1])
                    nc.sync.dma_start(dst_dram[128 * w:128 * (w + 1), :],
                                      fl[:])
                    del ps[w]
        ztile = self.z16_t
        for w in range(st.nw):
            if not st.has_any[w]:
                nc.sync.dma_start(dst_dram[128 * w:128 * (w + 1), :],
                                  ztile[:])

    def emit_linear(self, name, sources, Ws, bias_t, scale_t, dst_dram,
                    nchunks):
        """dst chunk = (sum_s srcT_chunk_s.T @ Ws[s] + bias) * scale.

        sources: list of ('hostT', dram [128, n]) | ('dramT', dram [n, 128])
                 | ('gatherT', dram, idx_dram)."""
        nc, mybir = self.nc, self.mybir
        f32, f16, i16 = mybir.dt.float32, mybir.dt.float16, mybir.dt.int16
        act = mybir.ActivationFunctionType
        gtiles = {}

        def srcT(si, i, spec):
            kind = spec[0]
            if kind == "hostT":
                t = self.p_lin.tile([128, 128], f16, tag="lt")
                nc.sync.dma_start(t[:], spec[1][:, 128 * i:128 * (i + 1)])
                return t[:]
            if kind == "dramT":
                t = self.p_lin.tile([128, 128], f16, tag="lt")
                nc.sync.dma_start(t[:], spec[1][128 * i:128 * (i + 1), :],
                                  transpose=True)
                return t[:]
            grp = i // GCH
            if (si, grp) not in gtiles:
                n_in = min(GCH, nchunks - grp * GCH)
                idx_t = self.p_meta.tile([128, n_in * 8], i16, tag="lidx")
                nc.sync.dma_start(
                    idx_t[:],
                    spec[2][:, grp * GCH * 8:(grp * GCH + n_in) * 8])
                g_t = self.p_gath.tile([128, 1, n_in * 128], f16, tag="lg")
                nc.gpsimd.dma_gather(
                    g_t[:], spec[1][:, :], idx_t[:],
                    num_idxs=n_in * 128, num_idxs_reg=n_in * 128,
                    elem_size=128, transpose=True,
                    single_packet=False, queue_num=self.next_q())
                gtiles[(si, grp)] = g_t
            return gtiles[(si, grp)][:, 0, 128 * (i % GCH):128 * (i % GCH + 1)]

        for i in range(nchunks):
            ps = self.p_ps2.tile([128, 128], f32, tag="lin")
            for si, spec in enumerate(sources):
                nc.tensor.matmul(ps[:], srcT(si, i, spec), Ws[si][:],
                                 start=(si == 0), stop=False)
            nc.tensor.matmul(ps[:], self.ones1_t[:], bias_t[:],
                             start=False, stop=True)
            fl = self.p_fl.tile([128, 128], f16, tag="lfl")
            nc.scalar.activation(fl[:], ps[:], act.Copy,
                                 scale=scale_t[:, i:i + 1])
            nc.sync.dma_start(dst_dram[128 * i:128 * (i + 1), :], fl[:])


# ---------------------------------------------------------------- main
def prepare(inputs):
    """Host-side schedule construction (numpy only, no device imports)."""
    H = [
        (np.asarray(inputs["H0_v"]).astype(np.int64),
         np.asarray(inputs["H0_e"]).astype(np.int64), N0, E0),
        (np.asarray(inputs["H1_v"]).astype(np.int64),
         np.asarray(inputs["H1_e"]).astype(np.int64), N1, E1),
        (np.asarray(inputs["H2_v"]).astype(np.int64),
         np.asarray(inputs["H2_e"]).astype(np.int64), N2, E2),
    ]
    assign0 = np.asarray(inputs["assign0"]).astype(np.int64)
    assign1 = np.asarray(inputs["assign1"]).astype(np.int64)

    n0l, n1l, n2l = _pad_local(N0), _pad_local(N1), _pad_local(N2)
    e0p, e1p, e2p = _pad128(E0), _pad128(E1), _pad128(E2)
    P = {"n0l": n0l, "n1l": n1l, "n2l": n2l,
         "e0p": e0p, "e1p": e1p, "e2p": e2p}

    def lap(lv, nloc):
        vi, ei, n, e = H[lv]
        dv_is, de_i = _degrees(vi, ei, n, e)
        owner, slot = vi % C, vi // C
        s1s, s1t, s2s, s2t = [], [], [], []
        for c in range(C):
            m = owner == c
            s1s.append(slot[m]); s1t.append(ei[m])
            s2s.append(ei[m]); s2t.append(slot[m])
        ep = _pad128(e)
        st1 = SegStage(f"l{lv}s1", s1s, s1t, nloc, ep)
        st2 = SegStage(f"l{lv}s2", s2s, s2t, ep, nloc)
        # scale arrays: [128, nw] column-per-window (f32)
        de_pad = np.zeros(ep, np.float32); de_pad[:e] = de_i[:e]
        sc1 = np.ascontiguousarray(de_pad.reshape(-1, 128).T)
        sc2 = []
        for c in range(C):
            g = np.arange(c, n, C)
            dv_loc = np.zeros(nloc, np.float32)
            dv_loc[: len(g)] = dv_is[g]
            sc2.append(np.ascontiguousarray(dv_loc.reshape(-1, 128).T))
        return st1, st2, sc1, sc2

    st1_0, st2_0, de0_sc, dv0_sc = lap(0, n0l)
    st1_1, st2_1, de1_sc, dv1_sc = lap(1, n1l)
    st1_2, st2_2, de2_sc, dv2_sc = lap(2, n2l)

    def pool(name, assign, nfine, ncoarse, ncl):
        cnt = np.bincount(assign, minlength=ncoarse).astype(np.float32)
        inv = np.where(cnt > 0, 1.0 / cnt, 0.0).astype(np.float32)
        srcs, tgts = [], []
        for c in range(C):
            g = np.arange(c, nfine, C)
            a = assign[g]
            srcs.append(g // C)
            tgts.append((a % C) * ncl + a // C)
        nfl = _pad_local(nfine)
        st = SegStage(name, srcs, tgts, nfl, C * ncl)
        # inv over P rows: r -> cluster a = (r % ncl)*C + r//ncl
        rows = np.arange(C * ncl)
        a = (rows % ncl) * C + rows // ncl
        sc = np.where(a < ncoarse, inv[np.minimum(a, ncoarse - 1)], 0.0)
        sc = np.ascontiguousarray(sc.astype(np.float32).reshape(-1, 128).T)
        return st, sc

    pool0, inv1_sc = pool("pool0", assign0, N0, N1, n1l)
    pool1, inv2_sc = pool("pool1", assign1, N1, N2, n2l)

    def unpool_idx(assign, nfine, ncl, nchunks):
        idxs = []
        for c in range(C):
            a = assign[np.arange(c, nfine, C)]
            idxs.append((a % C) * ncl + a // C)
        return _lin_gather_idx(idxs, nchunks)

    up1_arrs, up1_planes = unpool_idx(assign1, N1, n2l, n1l // 128)
    up0_arrs, up0_planes = unpool_idx(assign0, N0, n1l, n0l // 128)

    # host-expanded X in st1_0 slot order (fused lin0+stage1, no gather):
    # col s of xexpT = dv0[v(s)] * X[v(s)], transposed [128, L]; dvb = dv0[v(s)]
    X = np.asarray(inputs["X"], np.float32)
    vi0 = H[0][0]
    dv_is0, _ = _degrees(H[0][0], H[0][1], N0, E0)
    xexpT, dvb = [], []
    for c in range(C):
        g = np.arange(c, N0, C)
        xl = np.zeros((n0l + 128, D_IN), np.float32)
        xl[: len(g)] = X[g] * dv_is0[g][:, None]
        dvl = np.zeros(n0l + 128, np.float32)
        dvl[: len(g)] = dv_is0[g]
        idx = st1_0.idx_arrs[c]
        xexpT.append(np.ascontiguousarray(xl[idx].T.astype(np.float16)))
        dvb.append(np.ascontiguousarray(dvl[idx].astype(np.float16)
                                        .reshape(1, -1)))

    W = {k: np.asarray(inputs[k], np.float32) for k in
         ("W0", "W1", "W2", "W3", "W4", "b0", "b1", "b2", "b3", "b4")}
    return dict(P=P, stages=dict(
        st1_0=st1_0, st2_0=st2_0, st1_1=st1_1, st2_1=st2_1,
        st1_2=st1_2, st2_2=st2_2, pool0=pool0, pool1=pool1),
        scales=dict(de0=de0_sc, dv0=dv0_sc, de1=de1_sc, dv1=dv1_sc,
                    de2=de2_sc, dv2=dv2_sc, inv1=inv1_sc, inv2=inv2_sc),
        up0=(up0_arrs, up0_planes), up1=(up1_arrs, up1_planes),
        xexpT=xexpT, dvb=dvb, W=W)


def emulate(prep, inputs):
    """Pure-numpy replay of the device schedule (fp32 math)."""
    P = prep["P"]; S = prep["stages"]; SC = prep["scales"]; W = prep["W"]
    n0l, n1l, n2l = P["n0l"], P["n1l"], P["n2l"]
    e0p, e1p, e2p = P["e0p"], P["e1p"], P["e2p"]

    def f16(a):
        return a.astype(np.float16).astype(np.float32)

    def lin(sources_T, Ws, b, scale_cols, rows):
        outs = []
        for c in range(C):
            acc = sum(sT.T @ w for sT, w in zip(sources_T[c], Ws)) + b
            sc = scale_cols[c] if isinstance(scale_cols, list) else scale_cols
            acc = acc * sc.T.reshape(-1)[:rows, None]
            outs.append(f16(acc))
        return outs

    def with_z(tabs, pad_rows=128):
        return [np.vstack([t, np.zeros((pad_rows, t.shape[1]), np.float32)])
                for t in tabs]

    # fused lin0+st1_0: per-slot rows from xexpT, same block schedule
    st10 = S["st1_0"]
    W0q = f16(W["W0"])
    Y0p = []
    for c in range(C):
        rows = f16(prep["xexpT"][c].astype(np.float32).T @ W0q
                   + prep["dvb"][c].astype(np.float32).T * W["b0"])
        scflat = SC["de0"].T.reshape(-1)
        out = np.zeros((st10.nw * 128, 128), np.float32)
        ps = {}
        for i, (w, kind, j) in enumerate(st10.blocks):
            g = rows[i * 128:(i + 1) * 128]
            if w not in ps:
                ps[w] = np.zeros((128, 128), np.float32)
            if kind == 0:
                ps[w] += g
            else:
                ps[w] += st10.oh_arrs[c][j].astype(np.float32).T @ g
            if st10.first_last[i][1]:
                out[w * 128:(w + 1) * 128] = \
                    ps.pop(w) * scflat[w * 128:(w + 1) * 128, None]
        Y0p.append(out)
    Y0f = [f16(sum(Y0p))] * C
    h0 = S["st2_0"].emulate(with_z(Y0f), n0l, SC["dv0"], True)
    h0 = [f16(t) for t in h0]
    P1p = S["pool0"].emulate(with_z(h0), C * n1l, [SC["inv1"]] * C, False)
    P1s_full = sum(P1p)
    P1s = [f16(P1s_full[c * n1l:(c + 1) * n1l]) for c in range(C)]
    T1 = lin([[P1s[c].T] for c in range(C)], [f16(W["W1"])], W["b1"],
             SC["dv1"], n1l)
    Y1p = S["st1_1"].emulate(with_z(T1), e1p, [SC["de1"]] * C, False)
    Y1f = [f16(sum(Y1p))] * C
    h1 = S["st2_1"].emulate(with_z(Y1f), n1l, SC["dv1"], True)
    h1 = [f16(t) for t in h1]
    P2p = S["pool1"].emulate(with_z(h1), C * n2l, [SC["inv2"]] * C, False)
    P2s_full = sum(P2p)
    P2s = [f16(P2s_full[c * n2l:(c + 1) * n2l]) for c in range(C)]
    T2 = lin([[P2s[c].T] for c in range(C)], [f16(W["W2"])], W["b2"],
             SC["dv2"], n2l)
    Y2p = S["st1_2"].emulate(with_z(T2), e2p, [SC["de2"]] * C, False)
    Y2f = [f16(sum(Y2p))] * C
    Xc2 = S["st2_2"].emulate(with_z(Y2f), n2l, SC["dv2"], True)
    Xc2 = [f16(t) for t in Xc2]
    Xc2f = np.vstack(Xc2)  # [C*n2l, 128]
    Xc2fz = np.vstack([Xc2f, np.zeros((128, 128), np.float32)])
    up0_arrs, _ = prep["up0"]; up1_arrs, _ = prep["up1"]
    W3 = f16(W["W3"]); W4 = f16(W["W4"])
    T3 = lin([[Xc2fz[up1_arrs[c]].T, h1[c].T] for c in range(C)],
             [W3[:128], W3[128:]], W["b3"], SC["dv1"], n1l)
    Y3p = S["st1_1"].emulate(with_z(T3), e1p, [SC["de1"]] * C, False)
    Y3f = [f16(sum(Y3p))] * C
    Xu1 = S["st2_1"].emulate(with_z(Y3f), n1l, SC["dv1"], True)
    Xu1 = [f16(t) for t in Xu1]
    Xuf = np.vstack(Xu1)
    Xufz = np.vstack([Xuf, np.zeros((128, 128), np.float32)])
    W4p = np.zeros((256, 128), np.float32); W4p[:, :64] = W4
    b4p = np.zeros(128, np.float32); b4p[:64] = W["b4"]
    T4 = lin([[Xufz[up0_arrs[c]].T, h0[c].T] for c in range(C)],
             [W4p[:128], W4p[128:]], b4p, SC["dv0"], n0l)
    Y4p = S["st1_0"].emulate(with_z(T4), e0p, [SC["de0"]] * C, False)
    Y4f = [f16(sum(Y4p))] * C
    outs = S["st2_0"].emulate(with_z(Y4f), n0l, SC["dv0"], False, cols=64)
    out = np.empty((N0, D_OUT), np.float32)
    for c in range(C):
        n = len(range(c, N0, C))
        out[c::C] = outs[c][:n]
    return out


def build(prep):
    import concourse.bass as bass  # noqa: F401
    import concourse.tile as tile
    from concourse import bacc, mybir
    from contextlib import ExitStack

    P = prep["P"]; S = prep["stages"]; SC = prep["scales"]; W = prep["W"]
    n0l, n1l, n2l = P["n0l"], P["n1l"], P["n2l"]
    e0p, e1p, e2p = P["e0p"], P["e1p"], P["e2p"]

    nc = bacc.Bacc("TRN2", target_bir_lowering=False, debug=False,
                   num_devices=C, num_swdge_queues=NQ)
    f32, f16, i16 = mybir.dt.float32, mybir.dt.float16, mybir.dt.int16
    B = Builder(nc, mybir)

    def dram(name, rows, d=128, dt=None, shared=False):
        return nc.dram_tensor(name, [rows, d], dt or f16,
                              addr_space="Shared" if shared else "Local")

    # tables (+128 zero window on every gather source)
    Y0p = dram("Y0p", e0p); Y0f = dram("Y0f", e0p + 128, shared=True)
    h0 = dram("h0", n0l + 128)
    P1p = dram("P1p", C * n1l); P1s = dram("P1s", n1l)
    T1 = dram("T1", n1l + 128)
    Y1p = dram("Y1p", e1p); Y1f = dram("Y1f", e1p + 128, shared=True)
    h1 = dram("h1", n1l + 128)
    P2p = dram("P2p", C * n2l); P2s = dram("P2s", n2l)
    T2 = dram("T2", n2l + 128)
    Y2p = dram("Y2p", e2p); Y2f = dram("Y2f", e2p + 128, shared=True)
    Xc2 = dram("Xc2", n2l); Xc2f = dram("Xc2f", C * n2l + 128, shared=True)
    T3 = dram("T3", n1l + 128)
    Y3p = dram("Y3p", e1p); Y3f = dram("Y3f", e1p + 128, shared=True)
    Xu1 = dram("Xu1", n1l); Xuf = dram("Xuf", C * n1l + 128, shared=True)
    T4 = dram("T4", n0l + 128)
    Y4p = dram("Y4p", e0p); Y4f = dram("Y4f", e0p + 128, shared=True)
    out_d = nc.dram_tensor("out", [n0l, D_OUT], f32, kind="ExternalOutput")

    L10 = S["st1_0"].L
    xexpT_d = B.add_input("xexpT", [128, L10], f16, prep["xexpT"])
    dvb_d = B.add_input("dvb", [1, L10], f16, prep["dvb"])
    up0_d = B.add_input("up0_idx", list(prep["up0"][1][0].shape), i16,
                        prep["up0"][1])
    up1_d = B.add_input("up1_idx", list(prep["up1"][1][0].shape), i16,
                        prep["up1"][1])

    rg = [list(range(C))]

    def coll(kind, src_ap, dst_ap):
        op = mybir.AluOpType.bypass if kind == "AllGather" else \
            mybir.AluOpType.add
        nc.gpsimd.collective_compute(
            kind, op, replica_groups=rg,
            ins=[src_ap.opt()], outs=[dst_ap.opt()])

    with ExitStack() as ctx:
        tc = ctx.enter_context(tile.TileContext(nc))
        B.tc = tc
        B.setup_pools(ctx)
        B.setup_consts()
        W0t = B.const_mat("w0", W["W0"].astype(np.float16))
        W1t = B.const_mat("w1", W["W1"].astype(np.float16))
        W2t = B.const_mat("w2", W["W2"].astype(np.float16))
        W3a = B.const_mat("w3a", W["W3"][:128].astype(np.float16))
        W3b = B.const_mat("w3b", W["W3"][128:].astype(np.float16))
        W4pad = np.zeros((256, 128), np.float16)
        W4pad[:, :64] = W["W4"].astype(np.float16)
        W4a = B.const_mat("w4a", W4pad[:128])
        W4b = B.const_mat("w4b", W4pad[128:])
        b4pad = np.zeros(128, np.float32); b4pad[:64] = W["b4"]
        bts = {}
        for k, v in (("b0", W["b0"]), ("b1", W["b1"]), ("b2", W["b2"]),
                     ("b3", W["b3"]), ("b4", b4pad)):
            bts[k] = B.const_mat(k, v.astype(np.float16).reshape(1, 128))
        scs = {}
        for k in ("de0", "de1", "de2", "inv1", "inv2"):
            scs[k] = B.const_mat("sc_" + k, SC[k])
        for k in ("dv0", "dv1", "dv2"):
            scs[k] = B.const_mat("sc_" + k, SC[k])

        # zero windows for all gather-source tables
        for t, rows in ((T1, n1l), (T2, n2l), (T3, n1l),
                        (T4, n0l), (Y0f, e0p), (Y1f, e1p), (Y2f, e2p),
                        (Y3f, e1p), (Y4f, e0p), (h0, n0l), (h1, n1l),
                        (Xc2f, C * n2l), (Xuf, C * n1l)):
            nc.sync.dma_start(t[rows:rows + 128, :], B.z16_t[:])

        B.emit_fused_stage(S["st1_0"], xexpT_d, dvb_d, W0t, bts["b0"],
                           scs["de0"], Y0p)
        coll("AllReduce", Y0p.ap(), Y0f[0:e0p, :])
        B.emit_stage(S["st2_0"], Y0f, h0, scs["dv0"], True)
        B.emit_stage(S["pool0"], h0, P1p, scs["inv1"], False)
        coll("ReduceScatter", P1p.ap(), P1s.ap())
        B.emit_linear("lin1", [("dramT", P1s)], [W1t], bts["b1"],
                      scs["dv1"], T1, n1l // 128)
        B.emit_stage(S["st1_1"], T1, Y1p, scs["de1"], False)
        coll("AllReduce", Y1p.ap(), Y1f[0:e1p, :])
        B.emit_stage(S["st2_1"], Y1f, h1, scs["dv1"], True)
        B.emit_stage(S["pool1"], h1, P2p, scs["inv2"], False)
        coll("ReduceScatter", P2p.ap(), P2s.ap())
        B.emit_linear("lin2", [("dramT", P2s)], [W2t], bts["b2"],
                      scs["dv2"], T2, n2l // 128)
        B.emit_stage(S["st1_2"], T2, Y2p, scs["de2"], False)
        coll("AllReduce", Y2p.ap(), Y2f[0:e2p, :])
        B.emit_stage(S["st2_2"], Y2f, Xc2, scs["dv2"], True)
        coll("AllGather", Xc2.ap(), Xc2f[0:C * n2l, :])
        B.emit_linear("lin3", [("gatherT", Xc2f, up1_d), ("dramT", h1)],
                      [W3a, W3b], bts["b3"], scs["dv1"], T3, n1l // 128)
        B.emit_stage(S["st1_1"].renamed("l1bs1"), T3, Y3p, scs["de1"], False)
        coll("AllReduce", Y3p.ap(), Y3f[0:e1p, :])
        B.emit_stage(S["st2_1"].renamed("l1bs2"), Y3f, Xu1, scs["dv1"], True)
        coll("AllGather", Xu1.ap(), Xuf[0:C * n1l, :])
        B.emit_linear("lin4", [("gatherT", Xuf, up0_d), ("dramT", h0)],
                      [W4a, W4b], bts["b4"], scs["dv0"], T4, n0l // 128)
        B.emit_stage(S["st1_0"].renamed("l0bs1"), T4, Y4p, scs["de0"], False)
        coll("AllReduce", Y4p.ap(), Y4f[0:e0p, :])
        B.emit_stage(S["st2_0"].renamed("l0bs2"), Y4f, out_d, scs["dv0"],
                     False, out_dtype=f32, cols=D_OUT)
    nc.compile()

    in_maps = []
    for c in range(C):
        m = {}
        for name, arrs in B.inputs.items():
            m[name] = arrs[c] if isinstance(arrs, list) else arrs
        in_maps.append(m)
    return nc, in_maps


LAST_EXEC_NS = None


def _install_ntff_hook():
    import contextlib, ctypes, types
    try:
        from antenv import axon_hooks  # noqa: F401
        return
    except ImportError:
        pass
    import antenv
    so_path = os.environ.get("PJRT_LIBRARY_PATH", "/opt/axon/libaxon_pjrt.so")
    try:
        lib = ctypes.CDLL(so_path)
    except OSError:
        lib = None
    hook = None
    if lib is not None and hasattr(lib, "axon_start_nrt_profile"):
        lib.axon_start_nrt_profile.argtypes = [
            ctypes.POINTER(ctypes.c_int64), ctypes.c_size_t]
        lib.axon_start_nrt_profile.restype = ctypes.c_int64
        lib.axon_stop_nrt_profile.argtypes = [ctypes.c_char_p]
        lib.axon_stop_nrt_profile.restype = ctypes.c_int64

        @contextlib.contextmanager
        def hook(output_dir, device_ids):
            import jax
            jax.devices()
            if device_ids:
                ids = (ctypes.c_int64 * len(device_ids))(*device_ids)
                rc = lib.axon_start_nrt_profile(ids, len(device_ids))
            else:
                rc = lib.axon_start_nrt_profile(None, 0)
            if rc != 0:
                raise RuntimeError(f"axon_start_nrt_profile rc={rc}")
            try:
                yield
            finally:
                lib.axon_stop_nrt_profile(str(output_dir).encode())

    mod = types.ModuleType("antenv.axon_hooks")
    mod._hook = hook
    mod.get_axon_ntff_profile_hook = lambda: mod._hook

    def _set(h):
        mod._hook = h
    mod.set_axon_ntff_profile_hook = _set
    sys.modules["antenv.axon_hooks"] = mod
    antenv.axon_hooks = mod


def kernel(**inputs):
    global LAST_EXEC_NS
    prep = prepare(inputs)
    if os.environ.get("HGNN_EMULATE", "0") == "1":
        return emulate(prep, inputs)
    trace = os.environ.get("HGNN_TRACE", "0") == "1"
    if trace:
        _install_ntff_hook()
    nc, in_maps = build(prep)
    from concourse.bass_utils import run_bass_kernel_spmd
    res = run_bass_kernel_spmd(nc, in_maps, core_ids=list(range(C)),
                               trace=trace)
    LAST_EXEC_NS = res.exec_time_ns
    out = np.empty((N0, D_OUT), np.float32)
    for c in range(C):
        n = len(range(c, N0, C))
        out[c::C] = res.results[c]["out"][:n]
    return out


# revision 16
# speedup vs baseline: 2.0132x; 1.3058x over previous
"""Trainium2 Bass kernel for 3-level hierarchical hypergraph GNN (HGNN).

v2 design (8 NeuronCores, one SPMD NEFF, per-core index/one-hot data):
  - All feature tables fp16 (256B rows = dma_gather minimum element).
  - Segment-sum via identity-stationary matmuls: entries of each 128-row
    target window are layered so slot p of layer j holds the j-th entry
    targeting row p (pads gather a zero row). Overflow entries beyond the
    per-window layer count k_w go to host-precomputed 0/1 one-hot tail
    blocks (fp16, streamed from DRAM as lhsT).
  - All degree/pool normalizations folded into per-partition `scale` of
    the ACT-engine psum->SBUF flush (Relu fused where needed). The DVE is
    out of the hot path entirely (baseline bottleneck #1).
  - dma_gather descriptor generation (Q7, ~8ns/idx) spread over 4 SWDGE
    queues (num_swdge_queues=4, queue_num round-robin) -> 4 Q7 core pairs
    generate descriptors concurrently (baseline bottleneck #3: all on q0).
  - Linear layers: stationary = transposed input chunk (host-transposed X,
    HWDGE dma-transpose loads, or transpose-mode dma_gather for unpools),
    moving = weight; bias via rank-1 matmul; flush on ACT with dv scale.
  - Collectives in fp16 (half the bytes of the fp32 baseline).
"""
import sys

sys.path.insert(0, "/opt/trn_rl_repo")
import os
import numpy as np

C = 8
NQ = 4          # SWDGE queues to rotate over
CH = 32         # gather group size in 128-idx blocks
GCH = 16        # linear gather group in 128-row chunks
THETA = 0.85    # identity-layer utilization threshold
KMAX = 64

N0, N1, N2 = 100000, 25000, 6250
E0, E1, E2 = 20000, 5000, 1250
D_IN, D_H, D_OUT = 128, 128, 64


def _pad128(n):
    return ((n + 127) // 128) * 128


def _pad_local(n):
    return _pad128(-(-n // C))


# ---------------------------------------------------------------- host side
def _degrees(vi, ei, n, e):
    ones = np.ones(len(vi), np.float32)
    dV = np.bincount(vi, weights=ones, minlength=n)
    dE = np.bincount(ei, weights=ones, minlength=e)
    dv_is = np.where(dV > 0, dV ** -0.5, 0.0).astype(np.float32)
    de_i = np.where(dE > 0, 1.0 / dE, 0.0).astype(np.float32)
    return dv_is, de_i


def _plane_idx(idx):
    """int array (L,) -> [128, L//16] int16 (16-partition wrap, replicated x8)."""
    assert len(idx) % 16 == 0
    assert idx.max(initial=0) < 32768
    return np.tile(idx.astype(np.int16).reshape(-1, 16).T, (C, 1)).copy()


class SegStage:
    """Identity-layer + one-hot-tail schedule for one segment-sum stage.

    srcs/tgts: per-core int arrays; src = row in source table (local),
    tgt = row in destination table. n_src_pad = zero-row index in source.
    """

    def __init__(self, name, srcs, tgts, n_src_pad, n_tgt_pad, theta=THETA):
        self.name = name
        self.zrow = n_src_pad
        nw = n_tgt_pad // 128
        self.nw = nw

        # per-core per-target counts and entry occurrence ranks
        cnts = np.zeros((C, n_tgt_pad), np.int64)
        occs, orders = [], []
        for c in range(C):
            t = tgts[c]
            cnts[c] = np.bincount(t, minlength=n_tgt_pad)
            order = np.argsort(t, kind="stable")
            st = t[order]
            # occurrence rank within equal-target runs
            grp_start = np.r_[0, np.flatnonzero(np.diff(st)) + 1]
            run_len = np.diff(np.r_[grp_start, len(st)])
            occ_sorted = np.arange(len(st)) - np.repeat(grp_start, run_len)
            occ = np.empty(len(st), np.int64)
            occ[order] = occ_sorted
            occs.append(occ)
            orders.append(order)

        # pooled layer utilization per window -> k_w
        cap = np.minimum(cnts, KMAX)
        U = np.zeros((nw, KMAX + 1), np.int64)
        wid = np.arange(n_tgt_pad) // 128
        for c in range(C):
            np.add.at(U, (wid, cap[c]), 1)
        # ge[w, j] = #(core,target in w) with cnt >= j
        ge = U[:, ::-1].cumsum(axis=1)[:, ::-1]
        kws = np.zeros(nw, np.int64)
        for w in range(nw):
            k = 0
            while k < KMAX and ge[w, k + 1] >= theta * 128 * C:
                k += 1
            kws[w] = k
        self.kws = kws

        # tail block counts (uniform across cores)
        tails_per = np.zeros((C, nw), np.int64)
        for c in range(C):
            t = np.maximum(cnts[c] - kws[np.arange(n_tgt_pad) // 128], 0)
            tails_per[c] = np.bincount(wid, weights=t, minlength=nw).astype(np.int64)
        tbs = -(-tails_per.max(axis=0) // 128)
        # ensure windows with any entries produce blocks; empty windows get none
        has_any = (np.bincount(wid, weights=cnts.sum(axis=0), minlength=nw) > 0)
        self.tbs = np.where(has_any & (kws == 0) & (tbs == 0), 1, tbs)
        self.has_any = has_any

        # block list: per window, k_w identity blocks then tb_w one-hot blocks
        blocks = []  # (w, kind, layer_or_ohslot)
        oh_slot = 0
        for w in range(nw):
            if not has_any[w]:
                continue
            for j in range(kws[w]):
                blocks.append((w, 0, j))
            for i in range(self.tbs[w]):
                blocks.append((w, 1, oh_slot))
                oh_slot += 1
        self.blocks = blocks
        self.n_oh = oh_slot
        self.nblocks = len(blocks)
        self.L = 128 * self.nblocks

        # per-window block offset for vectorized fill
        blk_off = np.zeros(nw + 1, np.int64)
        for w in range(nw):
            blk_off[w + 1] = blk_off[w] + (kws[w] + self.tbs[w] if has_any[w] else 0)
        oh_off = np.zeros(nw, np.int64)
        s = 0
        for w in range(nw):
            oh_off[w] = s
            if has_any[w]:
                s += self.tbs[w]

        # build per-core idx streams and one-hot planes
        self.idx_arrs, self.oh_arrs = [], []
        for c in range(C):
            idx = np.full(self.L, self.zrow, np.int64)
            oh = np.zeros((max(self.n_oh, 1), 128, 128), np.float16)
            t = tgts[c]
            srcs_c = srcs[c]
            occ = occs[c]
            w_of = t // 128
            t_of = t % 128
            kw_of = kws[w_of]
            ident = occ < kw_of
            # identity entries: block = blk_off[w] + occ, slot = t_of
            b = blk_off[w_of[ident]] + occ[ident]
            idx[b * 128 + t_of[ident]] = srcs_c[ident]
            # tail entries: rank within (window) among tails, in stable order
            tm = ~ident
            tw = w_of[tm]
            order = np.argsort(tw, kind="stable")
            stw = tw[order]
            grp_start = np.r_[0, np.flatnonzero(np.diff(stw)) + 1]
            run_len = np.diff(np.r_[grp_start, len(stw)])
            rank_sorted = np.arange(len(stw)) - np.repeat(grp_start, run_len)
            rank = np.empty(len(stw), np.int64)
            rank[order] = rank_sorted
            tb = blk_off[tw] + kw_of[tm] + rank // 128
            ts = rank % 128
            idx[tb * 128 + ts] = srcs_c[tm]
            ohslot = oh_off[tw] + rank // 128
            oh[ohslot, ts, t_of[tm]] = 1.0
            self.idx_arrs.append(idx)
            self.oh_arrs.append(oh)

        self.idx_planes = [_plane_idx(a) for a in self.idx_arrs]
        if self.n_oh:
            # [128, n_oh*128]: block i cols [128i,128(i+1)), [slot p, target t]
            self.oh_planes = [
                np.ascontiguousarray(o.transpose(1, 0, 2).reshape(128, -1))
                for o in self.oh_arrs
            ]
        else:
            self.oh_planes = None

        # annotate first/last per window for psum start/stop
        self.first_last = []
        for i, (w, kind, j) in enumerate(blocks):
            first = i == 0 or blocks[i - 1][0] != w
            last = i == self.nblocks - 1 or blocks[i + 1][0] != w
            self.first_last.append((first, last))

    def renamed(self, name):
        st = SegStage.__new__(SegStage)
        st.__dict__ = dict(self.__dict__)
        st.name = name
        return st

    # ---- numpy emulation for self-test
    def emulate(self, src_tab, dst_rows, scale, relu, cols=128):
        """src_tab: per-core [n_src_pad+128, 128] f32. Returns per-core
        [nw*128, cols] f32 outputs (zeros for empty windows)."""
        outs = []
        for c in range(C):
            tab = src_tab[c]
            idx = self.idx_arrs[c]
            scflat = scale[c].T.reshape(-1)  # [128, nw] col-per-window -> flat
            out = np.zeros((self.nw * 128, cols), np.float32)
            ps = {}
            for i, (w, kind, j) in enumerate(self.blocks):
                g = tab[idx[i * 128:(i + 1) * 128]]
                if w not in ps:
                    ps[w] = np.zeros((128, 128), np.float32)
                if kind == 0:
                    ps[w] += g
                else:
                    ps[w] += self.oh_arrs[c][j].astype(np.float32).T @ g
                if self.first_last[i][1]:
                    r = ps.pop(w) * scflat[w * 128:(w + 1) * 128, None]
                    if relu:
                        r = np.maximum(r, 0.0)
                    out[w * 128:(w + 1) * 128] = r[:, :cols]
            outs.append(out)
        return outs


def _lin_gather_idx(idx_per_core, nchunks):
    """Pad per-core gather idx streams for linear sources to nchunks*128."""
    L = nchunks * 128
    planes = []
    arrs = []
    for c in range(C):
        gi = np.zeros(L, np.int64)
        gi[: len(idx_per_core[c])] = idx_per_core[c]
        arrs.append(gi)
        planes.append(_plane_idx(gi))
    return arrs, planes


# ---------------------------------------------------------------- device side
class Builder:
    def __init__(self, nc, mybir):
        self.nc = nc
        self.mybir = mybir
        self.tc = None
        self.inputs = {}
        self.qctr = 0

    def next_q(self):
        q = self.qctr % NQ
        self.qctr += 1
        return q

    def add_input(self, name, shape, dtype, arrays):
        assert name not in self.inputs, name
        t = self.nc.dram_tensor(name, list(shape), dtype, kind="ExternalInput")
        self.inputs[name] = arrays
        return t

    def setup_pools(self, ctx):
        tc = self.tc
        self.p_const = ctx.enter_context(tc.tile_pool(name="const", bufs=1))
        self.p_gath = ctx.enter_context(tc.tile_pool(name="gath", bufs=3))
        self.p_meta = ctx.enter_context(tc.tile_pool(name="meta", bufs=3))
        self.p_fl = ctx.enter_context(tc.tile_pool(name="fl", bufs=4))
        self.p_lin = ctx.enter_context(tc.tile_pool(name="lin", bufs=3))
        self.p_ps = ctx.enter_context(tc.tile_pool(name="ps", bufs=4, space="PSUM"))
        self.p_ps2 = ctx.enter_context(tc.tile_pool(name="ps2", bufs=2, space="PSUM"))

    def const_mat(self, name, arr, dtype=None):
        mybir = self.mybir
        a0 = arr[0] if isinstance(arr, list) else arr
        if dtype is None:
            dtype = mybir.dt.float32 if a0.dtype == np.float32 else mybir.dt.float16
        shape = list(a0.shape)
        d = self.add_input(name, shape, dtype, arr)
        t = self.p_const.tile(shape, dtype, tag=name)
        self.nc.sync.dma_start(t[:], d[:, :])
        return t

    def setup_consts(self):
        f16 = self.mybir.dt.float16
        self.ident_t = self.const_mat("c_ident", np.eye(128, dtype=np.float16))
        self.z16_t = self.const_mat("c_z16", np.zeros((128, 128), np.float16))
        self.z32_t = self.const_mat("c_z32", np.zeros((128, 128), np.float32))
        self.ones1_t = self.const_mat("c_ones1", np.ones((1, 128), np.float16))

    def emit_stage(self, st: SegStage, src_dram, dst_dram, scale_t, relu,
                   out_dtype=None, cols=128):
        nc, mybir = self.nc, self.mybir
        f32, f16, i16 = mybir.dt.float32, mybir.dt.float16, mybir.dt.int16
        if out_dtype is None:
            out_dtype = f16
        idx_d = self.add_input(f"{st.name}_idx", [128, st.L // 16], i16,
                               st.idx_planes)
        oh_d = None
        if st.n_oh:
            oh_d = self.add_input(f"{st.name}_oh", [128, st.n_oh * 128], f16,
                                  st.oh_planes)
        act = mybir.ActivationFunctionType
        func = act.Relu if relu else act.Copy

        ps = {}
        IDXB = 4
        idx_big, big_start = None, 0
        for start in range(0, st.nblocks, CH):
            nb = min(CH, st.nblocks - start)
            blks = st.blocks[start:start + nb]
            if idx_big is None or start - big_start >= IDXB * CH:
                span = min(IDXB * CH, st.nblocks - start)
                idx_big = self.p_meta.tile([128, span * 8], i16, tag="idx",
                                           name="idx_big")
                nc.sync.dma_start(idx_big[:],
                                  idx_d[:, start * 8:(start + span) * 8])
                big_start = start
            o8 = (start - big_start) * 8
            idx_t = idx_big[:, o8:o8 + nb * 8]
            ohs = [j for (_, kind, j) in blks if kind == 1]
            oh_t, oh0 = None, 0
            if ohs:
                oh0 = ohs[0]
                noh = ohs[-1] - oh0 + 1
                oh_t = self.p_meta.tile([128, noh * 128], f16, tag="oh")
                nc.sync.dma_start(oh_t[:],
                                  oh_d[:, oh0 * 128:(oh0 + noh) * 128])
            g_t = self.p_gath.tile([128, nb, 128], f16, tag="g")
            nc.gpsimd.dma_gather(
                g_t[:], src_dram[:, :], idx_t,
                num_idxs=nb * 128, num_idxs_reg=nb * 128, elem_size=128,
                single_packet=False, queue_num=self.next_q())
            for k, (w, kind, j) in enumerate(blks):
                first, last = st.first_last[start + k]
                if first:
                    ps[w] = self.p_ps.tile([128, 128], f32, tag="seg", name="seg_ps")
                lhsT = self.ident_t[:] if kind == 0 else \
                    oh_t[:, (j - oh0) * 128:(j - oh0 + 1) * 128]
                nc.tensor.matmul(ps[w][:], lhsT, g_t[:, k, :],
                                 start=first, stop=last)
                if last:
                    fl = self.p_fl.tile([128, cols], out_dtype, tag=f"fl{cols}")
                    nc.scalar.activation(fl[:], ps[w][:, 0:cols], func,
                                         scale=scale_t[:, w:w + 1])
                    nc.sync.dma_start(dst_dram[128 * w:128 * (w + 1), 0:cols],
                                      fl[:])
                    del ps[w]
        # zero-fill windows with no entries
        ztile = self.z16_t if out_dtype == f16 else self.z32_t
        for w in range(st.nw):
            if not st.has_any[w]:
                nc.sync.dma_start(dst_dram[128 * w:128 * (w + 1), 0:cols],
                                  ztile[:, 0:cols])

    def emit_fused_stage(self, st: SegStage, xexpT_d, dvb_d, W_t, brow_t,
                         scale_t, dst_dram):
        """st1_0 with lin0 fused: rhs tiles computed inline from host-expanded
        transposed input (slab loads, no gather)."""
        nc, mybir = self.nc, self.mybir
        f32, f16 = mybir.dt.float32, mybir.dt.float16
        act = mybir.ActivationFunctionType
        oh_d = None
        if st.n_oh:
            oh_d = self.add_input(f"{st.name}_oh", [128, st.n_oh * 128], f16,
                                  st.oh_planes)
        ps = {}
        for start in range(0, st.nblocks, CH):
            nb = min(CH, st.nblocks - start)
            blks = st.blocks[start:start + nb]
            slab = self.p_gath.tile([128, nb * 128], f16, tag="xslab")
            nc.sync.dma_start(slab[:],
                              xexpT_d[:, start * 128:(start + nb) * 128])
            dvb_t = self.p_meta.tile([1, nb * 128], f16, tag="dvb")
            nc.sync.dma_start(dvb_t[:],
                              dvb_d[:, start * 128:(start + nb) * 128])
            ohs = [j for (_, kind, j) in blks if kind == 1]
            oh_t, oh0 = None, 0
            if ohs:
                oh0 = ohs[0]
                noh = ohs[-1] - oh0 + 1
                oh_t = self.p_meta.tile([128, noh * 128], f16, tag="oh")
                nc.sync.dma_start(oh_t[:],
                                  oh_d[:, oh0 * 128:(oh0 + noh) * 128])
            for k, (w, kind, j) in enumerate(blks):
                first, last = st.first_last[start + k]
                ps_lin = self.p_ps2.tile([128, 128], f32, tag="flin",
                                         name="flin_ps")
                nc.tensor.matmul(ps_lin[:], slab[:, k * 128:(k + 1) * 128],
                                 W_t[:], start=True, stop=False)
                nc.tensor.matmul(ps_lin[:], dvb_t[:, k * 128:(k + 1) * 128],
                                 brow_t[:], start=False, stop=True)
                fl_lin = self.p_lin.tile([128, 128], f16, tag="flin_s")
                nc.vector.tensor_copy(fl_lin[:], ps_lin[:])
                if first:
                    ps[w] = self.p_ps.tile([128, 128], f32, tag="seg",
                                           name="seg_ps")
                lhsT = self.ident_t[:] if kind == 0 else \
                    oh_t[:, (j - oh0) * 128:(j - oh0 + 1) * 128]
                nc.tensor.matmul(ps[w][:], lhsT, fl_lin[:],
                                 start=first, stop=last)
                if last:
                    fl = self.p_fl.tile([128, 128], f16, tag="fl128")
                    nc.scalar.activation(fl[:], ps[w][:], act.Copy,
                                         scale=scale_t[:, w:w + 1])
                    nc.sync.dma_start(dst_dram[128 * w:128 * (w + 1), :],
                                      fl[:])
                    del ps[w]
        ztile = self.z16_t
        for w in range(st.nw):
            if not st.has_any[w]:
                nc.sync.dma_start(dst_dram[128 * w:128 * (w + 1), :],
                                  ztile[:])

    def emit_linear(self, name, sources, Ws, bias_t, scale_t, dst_dram,
                    nchunks):
        """dst chunk = (sum_s srcT_chunk_s.T @ Ws[s] + bias) * scale.

        sources: list of ('hostT', dram [128, n]) | ('dramT', dram [n, 128])
                 | ('gatherT', dram, idx_dram)."""
        nc, mybir = self.nc, self.mybir
        f32, f16, i16 = mybir.dt.float32, mybir.dt.float16, mybir.dt.int16
        act = mybir.ActivationFunctionType
        gtiles = {}

        def srcT(si, i, spec):
            kind = spec[0]
            if kind == "hostT":
                t = self.p_lin.tile([128, 128], f16, tag="lt")
                nc.sync.dma_start(t[:], spec[1][:, 128 * i:128 * (i + 1)])
                return t[:]
            if kind == "dramT":
                t = self.p_lin.tile([128, 128], f16, tag="lt")
                nc.sync.dma_start(t[:], spec[1][128 * i:128 * (i + 1), :],
                                  transpose=True)
                return t[:]
            grp = i // GCH
            if (si, grp) not in gtiles:
                n_in = min(GCH, nchunks - grp * GCH)
                idx_t = self.p_meta.tile([128, n_in * 8], i16, tag="lidx")
                nc.sync.dma_start(
                    idx_t[:],
                    spec[2][:, grp * GCH * 8:(grp * GCH + n_in) * 8])
                g_t = self.p_gath.tile([128, 1, n_in * 128], f16, tag="lg")
                nc.gpsimd.dma_gather(
                    g_t[:], spec[1][:, :], idx_t[:],
                    num_idxs=n_in * 128, num_idxs_reg=n_in * 128,
                    elem_size=128, transpose=True,
                    single_packet=False, queue_num=self.next_q())
                gtiles[(si, grp)] = g_t
            return gtiles[(si, grp)][:, 0, 128 * (i % GCH):128 * (i % GCH + 1)]

        for i in range(nchunks):
            ps = self.p_ps2.tile([128, 128], f32, tag="lin")
            for si, spec in enumerate(sources):
                nc.tensor.matmul(ps[:], srcT(si, i, spec), Ws[si][:],
                                 start=(si == 0), stop=False)
            nc.tensor.matmul(ps[:], self.ones1_t[:], bias_t[:],
                             start=False, stop=True)
            fl = self.p_fl.tile([128, 128], f16, tag="lfl")
            nc.scalar.activation(fl[:], ps[:], act.Copy,
                                 scale=scale_t[:, i:i + 1])
            nc.sync.dma_start(dst_dram[128 * i:128 * (i + 1), :], fl[:])


# ---------------------------------------------------------------- main
def prepare(inputs):
    """Host-side schedule construction (numpy only, no device imports)."""
    H = [
        (np.asarray(inputs["H0_v"]).astype(np.int64),
         np.asarray(inputs["H0_e"]).astype(np.int64), N0, E0),
        (np.asarray(inputs["H1_v"]).astype(np.int64),
         np.asarray(inputs["H1_e"]).astype(np.int64), N1, E1),
        (np.asarray(inputs["H2_v"]).astype(np.int64),
         np.asarray(inputs["H2_e"]).astype(np.int64), N2, E2),
    ]
    assign0 = np.asarray(inputs["assign0"]).astype(np.int64)
    assign1 = np.asarray(inputs["assign1"]).astype(np.int64)

    n0l, n1l, n2l = _pad_local(N0), _pad_local(N1), _pad_local(N2)
    e0p, e1p, e2p = _pad128(E0), _pad128(E1), _pad128(E2)
    P = {"n0l": n0l, "n1l": n1l, "n2l": n2l,
         "e0p": e0p, "e1p": e1p, "e2p": e2p}

    def lap(lv, nloc):
        vi, ei, n, e = H[lv]
        dv_is, de_i = _degrees(vi, ei, n, e)
        owner, slot = vi % C, vi // C
        s1s, s1t, s2s, s2t = [], [], [], []
        for c in range(C):
            m = owner == c
            s1s.append(slot[m]); s1t.append(ei[m])
            s2s.append(ei[m]); s2t.append(slot[m])
        ep = _pad128(e)
        st1 = SegStage(f"l{lv}s1", s1s, s1t, nloc, ep)
        st2 = SegStage(f"l{lv}s2", s2s, s2t, ep, nloc)
        # scale arrays: [128, nw] column-per-window (f32)
        de_pad = np.zeros(ep, np.float32); de_pad[:e] = de_i[:e]
        sc1 = np.ascontiguousarray(de_pad.reshape(-1, 128).T)
        sc2 = []
        for c in range(C):
            g = np.arange(c, n, C)
            dv_loc = np.zeros(nloc, np.float32)
            dv_loc[: len(g)] = dv_is[g]
            sc2.append(np.ascontiguousarray(dv_loc.reshape(-1, 128).T))
        return st1, st2, sc1, sc2

    st1_0, st2_0, de0_sc, dv0_sc = lap(0, n0l)
    st1_1, st2_1, de1_sc, dv1_sc = lap(1, n1l)
    st1_2, st2_2, de2_sc, dv2_sc = lap(2, n2l)

    def pool(name, assign, nfine, ncoarse, ncl):
        cnt = np.bincount(assign, minlength=ncoarse).astype(np.float32)
        inv = np.where(cnt > 0, 1.0 / cnt, 0.0).astype(np.float32)
        srcs, tgts = [], []
        for c in range(C):
            g = np.arange(c, nfine, C)
            a = assign[g]
            srcs.append(g // C)
            tgts.append((a % C) * ncl + a // C)
        nfl = _pad_local(nfine)
        st = SegStage(name, srcs, tgts, nfl, C * ncl)
        # inv over P rows: r -> cluster a = (r % ncl)*C + r//ncl
        rows = np.arange(C * ncl)
        a = (rows % ncl) * C + rows // ncl
        sc = np.where(a < ncoarse, inv[np.minimum(a, ncoarse - 1)], 0.0)
        sc = np.ascontiguousarray(sc.astype(np.float32).reshape(-1, 128).T)
        return st, sc

    pool0, inv1_sc = pool("pool0", assign0, N0, N1, n1l)
    pool1, inv2_sc = pool("pool1", assign1, N1, N2, n2l)

    def unpool_idx(assign, nfine, ncl, nchunks):
        idxs = []
        for c in range(C):
            a = assign[np.arange(c, nfine, C)]
            idxs.append((a % C) * ncl + a // C)
        return _lin_gather_idx(idxs, nchunks)

    up1_arrs, up1_planes = unpool_idx(assign1, N1, n2l, n1l // 128)
    up0_arrs, up0_planes = unpool_idx(assign0, N0, n1l, n0l // 128)

    # host-expanded X in st1_0 slot order (fused lin0+stage1, no gather):
    # col s of xexpT = dv0[v(s)] * X[v(s)], transposed [128, L]; dvb = dv0[v(s)]
    X = np.asarray(inputs["X"], np.float32)
    vi0 = H[0][0]
    dv_is0, _ = _degrees(H[0][0], H[0][1], N0, E0)
    xexpT, dvb = [], []
    for c in range(C):
        g = np.arange(c, N0, C)
        xl = np.zeros((n0l + 128, D_IN), np.float32)
        xl[: len(g)] = X[g] * dv_is0[g][:, None]
        dvl = np.zeros(n0l + 128, np.float32)
        dvl[: len(g)] = dv_is0[g]
        idx = st1_0.idx_arrs[c]
        xexpT.append(np.ascontiguousarray(xl[idx].T.astype(np.float16)))
        dvb.append(np.ascontiguousarray(dvl[idx].astype(np.float16)
                                        .reshape(1, -1)))

    W = {k: np.asarray(inputs[k], np.float32) for k in
         ("W0", "W1", "W2", "W3", "W4", "b0", "b1", "b2", "b3", "b4")}
    return dict(P=P, stages=dict(
        st1_0=st1_0, st2_0=st2_0, st1_1=st1_1, st2_1=st2_1,
        st1_2=st1_2, st2_2=st2_2, pool0=pool0, pool1=pool1),
        scales=dict(de0=de0_sc, dv0=dv0_sc, de1=de1_sc, dv1=dv1_sc,
                    de2=de2_sc, dv2=dv2_sc, inv1=inv1_sc, inv2=inv2_sc),
        up0=(up0_arrs, up0_planes), up1=(up1_arrs, up1_planes),
        xexpT=xexpT, dvb=dvb, W=W)


def emulate(prep, inputs):
    """Pure-numpy replay of the device schedule (fp32 math)."""
    P = prep["P"]; S = prep["stages"]; SC = prep["scales"]; W = prep["W"]
    n0l, n1l, n2l = P["n0l"], P["n1l"], P["n2l"]
    e0p, e1p, e2p = P["e0p"], P["e1p"], P["e2p"]

    def f16(a):
        return a.astype(np.float16).astype(np.float32)

    def lin(sources_T, Ws, b, scale_cols, rows):
        outs = []
        for c in range(C):
            acc = sum(sT.T @ w for sT, w in zip(sources_T[c], Ws)) + b
            sc = scale_cols[c] if isinstance(scale_cols, list) else scale_cols
            acc = acc * sc.T.reshape(-1)[:rows, None]
            outs.append(f16(acc))
        return outs

    def with_z(tabs, pad_rows=128):
        return [np.vstack([t, np.zeros((pad_rows, t.shape[1]), np.float32)])
                for t in tabs]

    # fused lin0+st1_0: per-slot rows from xexpT, same block schedule
    st10 = S["st1_0"]
    W0q = f16(W["W0"])
    Y0p = []
    for c in range(C):
        rows = f16(prep["xexpT"][c].astype(np.float32).T @ W0q
                   + prep["dvb"][c].astype(np.float32).T * W["b0"])
        scflat = SC["de0"].T.reshape(-1)
        out = np.zeros((st10.nw * 128, 128), np.float32)
        ps = {}
        for i, (w, kind, j) in enumerate(st10.blocks):
            g = rows[i * 128:(i + 1) * 128]
            if w not in ps:
                ps[w] = np.zeros((128, 128), np.float32)
            if kind == 0:
                ps[w] += g
            else:
                ps[w] += st10.oh_arrs[c][j].astype(np.float32).T @ g
            if st10.first_last[i][1]:
                out[w * 128:(w + 1) * 128] = \
                    ps.pop(w) * scflat[w * 128:(w + 1) * 128, None]
        Y0p.append(out)
    Y0f = [f16(sum(Y0p))] * C
    h0 = S["st2_0"].emulate(with_z(Y0f), n0l, SC["dv0"], True)
    h0 = [f16(t) for t in h0]
    P1p = S["pool0"].emulate(with_z(h0), C * n1l, [SC["inv1"]] * C, False)
    P1s_full = sum(P1p)
    P1s = [f16(P1s_full[c * n1l:(c + 1) * n1l]) for c in range(C)]
    T1 = lin([[P1s[c].T] for c in range(C)], [f16(W["W1"])], W["b1"],
             SC["dv1"], n1l)
    Y1p = S["st1_1"].emulate(with_z(T1), e1p, [SC["de1"]] * C, False)
    Y1f = [f16(sum(Y1p))] * C
    h1 = S["st2_1"].emulate(with_z(Y1f), n1l, SC["dv1"], True)
    h1 = [f16(t) for t in h1]
    P2p = S["pool1"].emulate(with_z(h1), C * n2l, [SC["inv2"]] * C, False)
    P2s_full = sum(P2p)
    P2s = [f16(P2s_full[c * n2l:(c + 1) * n2l]) for c in range(C)]
    T2 = lin([[P2s[c].T] for c in range(C)], [f16(W["W2"])], W["b2"],
             SC["dv2"], n2l)
    Y2p = S["st1_2"].emulate(with_z(T2), e2p, [SC["de2"]] * C, False)
    Y2f = [f16(sum(Y2p))] * C
    Xc2 = S["st2_2"].emulate(with_z(Y2f), n2l, SC["dv2"], True)
    Xc2 = [f16(t) for t in Xc2]
    Xc2f = np.vstack(Xc2)  # [C*n2l, 128]
    Xc2fz = np.vstack([Xc2f, np.zeros((128, 128), np.float32)])
    up0_arrs, _ = prep["up0"]; up1_arrs, _ = prep["up1"]
    W3 = f16(W["W3"]); W4 = f16(W["W4"])
    T3 = lin([[Xc2fz[up1_arrs[c]].T, h1[c].T] for c in range(C)],
             [W3[:128], W3[128:]], W["b3"], SC["dv1"], n1l)
    Y3p = S["st1_1"].emulate(with_z(T3), e1p, [SC["de1"]] * C, False)
    Y3f = [f16(sum(Y3p))] * C
    Xu1 = S["st2_1"].emulate(with_z(Y3f), n1l, SC["dv1"], True)
    Xu1 = [f16(t) for t in Xu1]
    Xuf = np.vstack(Xu1)
    Xufz = np.vstack([Xuf, np.zeros((128, 128), np.float32)])
    W4p = np.zeros((256, 128), np.float32); W4p[:, :64] = W4
    b4p = np.zeros(128, np.float32); b4p[:64] = W["b4"]
    T4 = lin([[Xufz[up0_arrs[c]].T, h0[c].T] for c in range(C)],
             [W4p[:128], W4p[128:]], b4p, SC["dv0"], n0l)
    Y4p = S["st1_0"].emulate(with_z(T4), e0p, [SC["de0"]] * C, False)
    Y4f = [f16(sum(Y4p))] * C
    outs = S["st2_0"].emulate(with_z(Y4f), n0l, SC["dv0"], False, cols=64)
    out = np.empty((N0, D_OUT), np.float32)
    for c in range(C):
        n = len(range(c, N0, C))
        out[c::C] = outs[c][:n]
    return out


def build(prep):
    import concourse.bass as bass  # noqa: F401
    import concourse.tile as tile
    from concourse import bacc, mybir
    from contextlib import ExitStack

    P = prep["P"]; S = prep["stages"]; SC = prep["scales"]; W = prep["W"]
    n0l, n1l, n2l = P["n0l"], P["n1l"], P["n2l"]
    e0p, e1p, e2p = P["e0p"], P["e1p"], P["e2p"]

    nc = bacc.Bacc("TRN2", target_bir_lowering=False, debug=False,
                   num_devices=C, num_swdge_queues=NQ)
    f32, f16, i16 = mybir.dt.float32, mybir.dt.float16, mybir.dt.int16
    B = Builder(nc, mybir)

    def dram(name, rows, d=128, dt=None, shared=False):
        return nc.dram_tensor(name, [rows, d], dt or f16,
                              addr_space="Shared" if shared else "Local")

    # tables (+128 zero window on every gather source)
    Y0p = dram("Y0p", e0p); Y0f = dram("Y0f", e0p + 128, shared=True)
    h0 = dram("h0", n0l + 128)
    P1p = dram("P1p", C * n1l); P1s = dram("P1s", n1l)
    T1 = dram("T1", n1l + 128)
    Y1p = dram("Y1p", e1p); Y1f = dram("Y1f", e1p + 128, shared=True)
    h1 = dram("h1", n1l + 128)
    P2p = dram("P2p", C * n2l); P2s = dram("P2s", n2l)
    T2 = dram("T2", n2l + 128)
    Y2p = dram("Y2p", e2p); Y2f = dram("Y2f", e2p + 128, shared=True)
    Xc2 = dram("Xc2", n2l); Xc2f = dram("Xc2f", C * n2l + 128, shared=True)
    T3 = dram("T3", n1l + 128)
    Y3p = dram("Y3p", e1p); Y3f = dram("Y3f", e1p + 128, shared=True)
    Xu1 = dram("Xu1", n1l); Xuf = dram("Xuf", C * n1l + 128, shared=True)
    T4 = dram("T4", n0l + 128)
    Y4p = dram("Y4p", e0p); Y4f = dram("Y4f", e0p + 128, shared=True)
    out_d = nc.dram_tensor("out", [n0l, D_OUT], f32, kind="ExternalOutput")

    L10 = S["st1_0"].L
    xexpT_d = B.add_input("xexpT", [128, L10], f16, prep["xexpT"])
    dvb_d = B.add_input("dvb", [1, L10], f16, prep["dvb"])
    up0_d = B.add_input("up0_idx", list(prep["up0"][1][0].shape), i16,
                        prep["up0"][1])
    up1_d = B.add_input("up1_idx", list(prep["up1"][1][0].shape), i16,
                        prep["up1"][1])

    rg = [list(range(C))]

    def coll(kind, src_ap, dst_ap):
        op = mybir.AluOpType.bypass if kind == "AllGather" else \
            mybir.AluOpType.add
        nc.gpsimd.collective_compute(
            kind, op, replica_groups=rg,
            ins=[src_ap.opt()], outs=[dst_ap.opt()])

    with ExitStack() as ctx:
        tc = ctx.enter_context(tile.TileContext(nc))
        B.tc = tc
        B.setup_pools(ctx)
        B.setup_consts()
        W0t = B.const_mat("w0", W["W0"].astype(np.float16))
        W1t = B.const_mat("w1", W["W1"].astype(np.float16))
        W2t = B.const_mat("w2", W["W2"].astype(np.float16))
        W3a = B.const_mat("w3a", W["W3"][:128].astype(np.float16))
        W3b = B.const_mat("w3b", W["W3"][128:].astype(np.float16))
        W4pad = np.zeros((256, 128), np.float16)
        W4pad[:, :64] = W["W4"].astype(np.float16)
        W4a = B.const_mat("w4a", W4pad[:128])
        W4b = B.const_mat("w4b", W4pad[128:])
        b4pad = np.zeros(128, np.float32); b4pad[:64] = W["b4"]
        bts = {}
        for k, v in (("b0", W["b0"]), ("b1", W["b1"]), ("b2", W["b2"]),
                     ("b3", W["b3"]), ("b4", b4pad)):
            bts[k] = B.const_mat(k, v.astype(np.float16).reshape(1, 128))
        scs = {}
        for k in ("de0", "de1", "de2", "inv1", "inv2"):
            scs[k] = B.const_mat("sc_" + k, SC[k])
        for k in ("dv0", "dv1", "dv2"):
            scs[k] = B.const_mat("sc_" + k, SC[k])

        # zero windows for all gather-source tables
        for t, rows in ((T1, n1l), (T2, n2l), (T3, n1l),
                        (T4, n0l), (Y0f, e0p), (Y1f, e1p), (Y2f, e2p),
                        (Y3f, e1p), (Y4f, e0p), (h0, n0l), (h1, n1l),
                        (Xc2f, C * n2l), (Xuf, C * n1l)):
            nc.sync.dma_start(t[rows:rows + 128, :], B.z16_t[:])

        B.emit_fused_stage(S["st1_0"], xexpT_d, dvb_d, W0t, bts["b0"],
                           scs["de0"], Y0p)
        coll("AllReduce", Y0p.ap(), Y0f[0:e0p, :])
        B.emit_stage(S["st2_0"], Y0f, h0, scs["dv0"], True)
        B.emit_stage(S["pool0"], h0, P1p, scs["inv1"], False)
        coll("ReduceScatter", P1p.ap(), P1s.ap())
        B.emit_linear("lin1", [("dramT", P1s)], [W1t], bts["b1"],
                      scs["dv1"], T1, n1l // 128)
        B.emit_stage(S["st1_1"], T1, Y1p, scs["de1"], False)
        coll("AllReduce", Y1p.ap(), Y1f[0:e1p, :])
        B.emit_stage(S["st2_1"], Y1f, h1, scs["dv1"], True)
        B.emit_stage(S["pool1"], h1, P2p, scs["inv2"], False)
        coll("ReduceScatter", P2p.ap(), P2s.ap())
        B.emit_linear("lin2", [("dramT", P2s)], [W2t], bts["b2"],
                      scs["dv2"], T2, n2l // 128)
        B.emit_stage(S["st1_2"], T2, Y2p, scs["de2"], False)
        coll("AllReduce", Y2p.ap(), Y2f[0:e2p, :])
        B.emit_stage(S["st2_2"], Y2f, Xc2, scs["dv2"], True)
        coll("AllGather", Xc2.ap(), Xc2f[0:C * n2l, :])
        B.emit_linear("lin3", [("gatherT", Xc2f, up1_d), ("dramT", h1)],
                      [W3a, W3b], bts["b3"], scs["dv1"], T3, n1l // 128)
        B.emit_stage(S["st1_1"].renamed("l1bs1"), T3, Y3p, scs["de1"], False)
        coll("AllReduce", Y3p.ap(), Y3f[0:e1p, :])
        B.emit_stage(S["st2_1"].renamed("l1bs2"), Y3f, Xu1, scs["dv1"], True)
        coll("AllGather", Xu1.ap(), Xuf[0:C * n1l, :])
        B.emit_linear("lin4", [("gatherT", Xuf, up0_d), ("dramT", h0)],
                      [W4a, W4b], bts["b4"], scs["dv0"], T4, n0l // 128)
        B.emit_stage(S["st1_0"].renamed("l0bs1"), T4, Y4p, scs["de0"], False)
        coll("AllReduce", Y4p.ap(), Y4f[0:e0p, :])
        B.emit_stage(S["st2_0"].renamed("l0bs2"), Y4f, out_d, scs["dv0"],
                     False, out_dtype=f32, cols=D_OUT)
    nc.compile()

    in_maps = []
    for c in range(C):
        m = {}
        for name, arrs in B.inputs.items():
            m[name] = arrs[c] if isinstance(arrs, list) else arrs
        in_maps.append(m)
    return nc, in_maps


LAST_EXEC_NS = None


def _install_ntff_hook():
    import contextlib, ctypes, types
    try:
        from antenv import axon_hooks  # noqa: F401
        return
    except ImportError:
        pass
    import antenv
    so_path = os.environ.get("PJRT_LIBRARY_PATH", "/opt/axon/libaxon_pjrt.so")
    try:
        lib = ctypes.CDLL(so_path)
    except OSError:
        lib = None
    hook = None
    if lib is not None and hasattr(lib, "axon_start_nrt_profile"):
        lib.axon_start_nrt_profile.argtypes = [
            ctypes.POINTER(ctypes.c_int64), ctypes.c_size_t]
        lib.axon_start_nrt_profile.restype = ctypes.c_int64
        lib.axon_stop_nrt_profile.argtypes = [ctypes.c_char_p]
        lib.axon_stop_nrt_profile.restype = ctypes.c_int64

        @contextlib.contextmanager
        def hook(output_dir, device_ids):
            import jax
            jax.devices()
            if device_ids:
                ids = (ctypes.c_int64 * len(device_ids))(*device_ids)
                rc = lib.axon_start_nrt_profile(ids, len(device_ids))
            else:
                rc = lib.axon_start_nrt_profile(None, 0)
            if rc != 0:
                raise RuntimeError(f"axon_start_nrt_profile rc={rc}")
            try:
                yield
            finally:
                lib.axon_stop_nrt_profile(str(output_dir).encode())

    mod = types.ModuleType("antenv.axon_hooks")
    mod._hook = hook
    mod.get_axon_ntff_profile_hook = lambda: mod._hook

    def _set(h):
        mod._hook = h
    mod.set_axon_ntff_profile_hook = _set
    sys.modules["antenv.axon_hooks"] = mod
    antenv.axon_hooks = mod


def kernel(**inputs):
    global LAST_EXEC_NS
    prep = prepare(inputs)
    if os.environ.get("HGNN_EMULATE", "0") == "1":
        return emulate(prep, inputs)
    trace = os.environ.get("HGNN_TRACE", "0") == "1"
    if trace:
        _install_ntff_hook()
    nc, in_maps = build(prep)
    from concourse.bass_utils import run_bass_kernel_spmd
    res = run_bass_kernel_spmd(nc, in_maps, core_ids=list(range(C)),
                               trace=trace)
    LAST_EXEC_NS = res.exec_time_ns
    out = np.empty((N0, D_OUT), np.float32)
    for c in range(C):
        n = len(range(c, N0, C))
        out[c::C] = res.results[c]["out"][:n]
    return out


# revision 19
# speedup vs baseline: 2.0170x; 1.0019x over previous
"""Trainium2 Bass kernel for 3-level hierarchical hypergraph GNN (HGNN).

v2 design (8 NeuronCores, one SPMD NEFF, per-core index/one-hot data):
  - All feature tables fp16 (256B rows = dma_gather minimum element).
  - Segment-sum via identity-stationary matmuls: entries of each 128-row
    target window are layered so slot p of layer j holds the j-th entry
    targeting row p (pads gather a zero row). Overflow entries beyond the
    per-window layer count k_w go to host-precomputed 0/1 one-hot tail
    blocks (fp16, streamed from DRAM as lhsT).
  - All degree/pool normalizations folded into per-partition `scale` of
    the ACT-engine psum->SBUF flush (Relu fused where needed). The DVE is
    out of the hot path entirely (baseline bottleneck #1).
  - dma_gather descriptor generation (Q7, ~8ns/idx) spread over 4 SWDGE
    queues (num_swdge_queues=4, queue_num round-robin) -> 4 Q7 core pairs
    generate descriptors concurrently (baseline bottleneck #3: all on q0).
  - Linear layers: stationary = transposed input chunk (host-transposed X,
    HWDGE dma-transpose loads, or transpose-mode dma_gather for unpools),
    moving = weight; bias via rank-1 matmul; flush on ACT with dv scale.
  - Collectives in fp16 (half the bytes of the fp32 baseline).
"""
import sys

sys.path.insert(0, "/opt/trn_rl_repo")
import os
import numpy as np

C = 8
NQ = 4          # SWDGE queues to rotate over
CH = 32         # gather group size in 128-idx blocks
GCH = 16        # linear gather group in 128-row chunks
THETA = 0.90    # identity-layer utilization threshold
KMAX = 64

N0, N1, N2 = 100000, 25000, 6250
E0, E1, E2 = 20000, 5000, 1250
D_IN, D_H, D_OUT = 128, 128, 64


def _pad128(n):
    return ((n + 127) // 128) * 128


def _pad_local(n):
    return _pad128(-(-n // C))


# ---------------------------------------------------------------- host side
def _degrees(vi, ei, n, e):
    ones = np.ones(len(vi), np.float32)
    dV = np.bincount(vi, weights=ones, minlength=n)
    dE = np.bincount(ei, weights=ones, minlength=e)
    dv_is = np.where(dV > 0, dV ** -0.5, 0.0).astype(np.float32)
    de_i = np.where(dE > 0, 1.0 / dE, 0.0).astype(np.float32)
    return dv_is, de_i


def _plane_idx(idx):
    """int array (L,) -> [128, L//16] int16 (16-partition wrap, replicated x8)."""
    assert len(idx) % 16 == 0
    assert idx.max(initial=0) < 32768
    return np.tile(idx.astype(np.int16).reshape(-1, 16).T, (C, 1)).copy()


class SegStage:
    """Identity-layer + one-hot-tail schedule for one segment-sum stage.

    srcs/tgts: per-core int arrays; src = row in source table (local),
    tgt = row in destination table. n_src_pad = zero-row index in source.
    """

    def __init__(self, name, srcs, tgts, n_src_pad, n_tgt_pad, theta=THETA):
        self.name = name
        self.zrow = n_src_pad
        nw = n_tgt_pad // 128
        self.nw = nw

        # per-core per-target counts and entry occurrence ranks
        cnts = np.zeros((C, n_tgt_pad), np.int64)
        occs, orders = [], []
        for c in range(C):
            t = tgts[c]
            cnts[c] = np.bincount(t, minlength=n_tgt_pad)
            order = np.argsort(t, kind="stable")
            st = t[order]
            # occurrence rank within equal-target runs
            grp_start = np.r_[0, np.flatnonzero(np.diff(st)) + 1]
            run_len = np.diff(np.r_[grp_start, len(st)])
            occ_sorted = np.arange(len(st)) - np.repeat(grp_start, run_len)
            occ = np.empty(len(st), np.int64)
            occ[order] = occ_sorted
            occs.append(occ)
            orders.append(order)

        # pooled layer utilization per window -> k_w
        cap = np.minimum(cnts, KMAX)
        U = np.zeros((nw, KMAX + 1), np.int64)
        wid = np.arange(n_tgt_pad) // 128
        for c in range(C):
            np.add.at(U, (wid, cap[c]), 1)
        # ge[w, j] = #(core,target in w) with cnt >= j
        ge = U[:, ::-1].cumsum(axis=1)[:, ::-1]
        kws = np.zeros(nw, np.int64)
        for w in range(nw):
            k = 0
            while k < KMAX and ge[w, k + 1] >= theta * 128 * C:
                k += 1
            kws[w] = k
        self.kws = kws

        # tail block counts (uniform across cores)
        tails_per = np.zeros((C, nw), np.int64)
        for c in range(C):
            t = np.maximum(cnts[c] - kws[np.arange(n_tgt_pad) // 128], 0)
            tails_per[c] = np.bincount(wid, weights=t, minlength=nw).astype(np.int64)
        tbs = -(-tails_per.max(axis=0) // 128)
        # ensure windows with any entries produce blocks; empty windows get none
        has_any = (np.bincount(wid, weights=cnts.sum(axis=0), minlength=nw) > 0)
        self.tbs = np.where(has_any & (kws == 0) & (tbs == 0), 1, tbs)
        self.has_any = has_any

        # block list: per window, k_w identity blocks then tb_w one-hot blocks
        blocks = []  # (w, kind, layer_or_ohslot)
        oh_slot = 0
        for w in range(nw):
            if not has_any[w]:
                continue
            for j in range(kws[w]):
                blocks.append((w, 0, j))
            for i in range(self.tbs[w]):
                blocks.append((w, 1, oh_slot))
                oh_slot += 1
        self.blocks = blocks
        self.n_oh = oh_slot
        self.nblocks = len(blocks)
        self.L = 128 * self.nblocks

        # per-window block offset for vectorized fill
        blk_off = np.zeros(nw + 1, np.int64)
        for w in range(nw):
            blk_off[w + 1] = blk_off[w] + (kws[w] + self.tbs[w] if has_any[w] else 0)
        oh_off = np.zeros(nw, np.int64)
        s = 0
        for w in range(nw):
            oh_off[w] = s
            if has_any[w]:
                s += self.tbs[w]

        # build per-core idx streams and one-hot planes
        self.idx_arrs, self.oh_arrs = [], []
        for c in range(C):
            idx = np.full(self.L, self.zrow, np.int64)
            oh = np.zeros((max(self.n_oh, 1), 128, 128), np.float16)
            t = tgts[c]
            srcs_c = srcs[c]
            occ = occs[c]
            w_of = t // 128
            t_of = t % 128
            kw_of = kws[w_of]
            ident = occ < kw_of
            # identity entries: block = blk_off[w] + occ, slot = t_of
            b = blk_off[w_of[ident]] + occ[ident]
            idx[b * 128 + t_of[ident]] = srcs_c[ident]
            # tail entries: rank within (window) among tails, in stable order
            tm = ~ident
            tw = w_of[tm]
            order = np.argsort(tw, kind="stable")
            stw = tw[order]
            grp_start = np.r_[0, np.flatnonzero(np.diff(stw)) + 1]
            run_len = np.diff(np.r_[grp_start, len(stw)])
            rank_sorted = np.arange(len(stw)) - np.repeat(grp_start, run_len)
            rank = np.empty(len(stw), np.int64)
            rank[order] = rank_sorted
            tb = blk_off[tw] + kw_of[tm] + rank // 128
            ts = rank % 128
            idx[tb * 128 + ts] = srcs_c[tm]
            ohslot = oh_off[tw] + rank // 128
            oh[ohslot, ts, t_of[tm]] = 1.0
            self.idx_arrs.append(idx)
            self.oh_arrs.append(oh)

        self.idx_planes = [_plane_idx(a) for a in self.idx_arrs]
        if self.n_oh:
            # [128, n_oh*128]: block i cols [128i,128(i+1)), [slot p, target t]
            self.oh_planes = [
                np.ascontiguousarray(o.transpose(1, 0, 2).reshape(128, -1))
                for o in self.oh_arrs
            ]
        else:
            self.oh_planes = None

        # annotate first/last per window for psum start/stop
        self.first_last = []
        for i, (w, kind, j) in enumerate(blocks):
            first = i == 0 or blocks[i - 1][0] != w
            last = i == self.nblocks - 1 or blocks[i + 1][0] != w
            self.first_last.append((first, last))

    def renamed(self, name):
        st = SegStage.__new__(SegStage)
        st.__dict__ = dict(self.__dict__)
        st.name = name
        return st

    # ---- numpy emulation for self-test
    def emulate(self, src_tab, dst_rows, scale, relu, cols=128):
        """src_tab: per-core [n_src_pad+128, 128] f32. Returns per-core
        [nw*128, cols] f32 outputs (zeros for empty windows)."""
        outs = []
        for c in range(C):
            tab = src_tab[c]
            idx = self.idx_arrs[c]
            scflat = scale[c].T.reshape(-1)  # [128, nw] col-per-window -> flat
            out = np.zeros((self.nw * 128, cols), np.float32)
            ps = {}
            for i, (w, kind, j) in enumerate(self.blocks):
                g = tab[idx[i * 128:(i + 1) * 128]]
                if w not in ps:
                    ps[w] = np.zeros((128, 128), np.float32)
                if kind == 0:
                    ps[w] += g
                else:
                    ps[w] += self.oh_arrs[c][j].astype(np.float32).T @ g
                if self.first_last[i][1]:
                    r = ps.pop(w) * scflat[w * 128:(w + 1) * 128, None]
                    if relu:
                        r = np.maximum(r, 0.0)
                    out[w * 128:(w + 1) * 128] = r[:, :cols]
            outs.append(out)
        return outs


def _lin_gather_idx(idx_per_core, nchunks):
    """Pad per-core gather idx streams for linear sources to nchunks*128."""
    L = nchunks * 128
    planes = []
    arrs = []
    for c in range(C):
        gi = np.zeros(L, np.int64)
        gi[: len(idx_per_core[c])] = idx_per_core[c]
        arrs.append(gi)
        planes.append(_plane_idx(gi))
    return arrs, planes


# ---------------------------------------------------------------- device side
class Builder:
    def __init__(self, nc, mybir):
        self.nc = nc
        self.mybir = mybir
        self.tc = None
        self.inputs = {}
        self.qctr = 0

    def next_q(self):
        q = self.qctr % NQ
        self.qctr += 1
        return q

    def add_input(self, name, shape, dtype, arrays):
        assert name not in self.inputs, name
        t = self.nc.dram_tensor(name, list(shape), dtype, kind="ExternalInput")
        self.inputs[name] = arrays
        return t

    def setup_pools(self, ctx):
        tc = self.tc
        self.p_const = ctx.enter_context(tc.tile_pool(name="const", bufs=1))
        self.p_gath = ctx.enter_context(tc.tile_pool(name="gath", bufs=3))
        self.p_meta = ctx.enter_context(tc.tile_pool(name="meta", bufs=3))
        self.p_fl = ctx.enter_context(tc.tile_pool(name="fl", bufs=4))
        self.p_lin = ctx.enter_context(tc.tile_pool(name="lin", bufs=3))
        self.p_ps = ctx.enter_context(tc.tile_pool(name="ps", bufs=4, space="PSUM"))
        self.p_ps2 = ctx.enter_context(tc.tile_pool(name="ps2", bufs=2, space="PSUM"))

    def const_mat(self, name, arr, dtype=None):
        mybir = self.mybir
        a0 = arr[0] if isinstance(arr, list) else arr
        if dtype is None:
            dtype = mybir.dt.float32 if a0.dtype == np.float32 else mybir.dt.float16
        shape = list(a0.shape)
        d = self.add_input(name, shape, dtype, arr)
        t = self.p_const.tile(shape, dtype, tag=name)
        self.nc.sync.dma_start(t[:], d[:, :])
        return t

    def setup_consts(self):
        f16 = self.mybir.dt.float16
        self.ident_t = self.const_mat("c_ident", np.eye(128, dtype=np.float16))
        self.z16_t = self.const_mat("c_z16", np.zeros((128, 128), np.float16))
        self.z32_t = self.const_mat("c_z32", np.zeros((128, 128), np.float32))
        self.ones1_t = self.const_mat("c_ones1", np.ones((1, 128), np.float16))

    def emit_stage(self, st: SegStage, src_dram, dst_dram, scale_t, relu,
                   out_dtype=None, cols=128):
        nc, mybir = self.nc, self.mybir
        f32, f16, i16 = mybir.dt.float32, mybir.dt.float16, mybir.dt.int16
        if out_dtype is None:
            out_dtype = f16
        idx_d = self.add_input(f"{st.name}_idx", [128, st.L // 16], i16,
                               st.idx_planes)
        oh_d = None
        if st.n_oh:
            oh_d = self.add_input(f"{st.name}_oh", [128, st.n_oh * 128], f16,
                                  st.oh_planes)
        act = mybir.ActivationFunctionType
        func = act.Relu if relu else act.Copy

        ps = {}
        IDXB = 6
        idx_big, big_start = None, 0
        for start in range(0, st.nblocks, CH):
            nb = min(CH, st.nblocks - start)
            blks = st.blocks[start:start + nb]
            if idx_big is None or start - big_start >= IDXB * CH:
                span = min(IDXB * CH, st.nblocks - start)
                idx_big = self.p_meta.tile([128, span * 8], i16, tag="idx",
                                           name="idx_big")
                nc.sync.dma_start(idx_big[:],
                                  idx_d[:, start * 8:(start + span) * 8])
                big_start = start
            o8 = (start - big_start) * 8
            idx_t = idx_big[:, o8:o8 + nb * 8]
            ohs = [j for (_, kind, j) in blks if kind == 1]
            oh_t, oh0 = None, 0
            if ohs:
                oh0 = ohs[0]
                noh = ohs[-1] - oh0 + 1
                oh_t = self.p_meta.tile([128, noh * 128], f16, tag="oh")
                nc.sync.dma_start(oh_t[:],
                                  oh_d[:, oh0 * 128:(oh0 + noh) * 128])
            g_t = self.p_gath.tile([128, nb, 128], f16, tag="g")
            nc.gpsimd.dma_gather(
                g_t[:], src_dram[:, :], idx_t,
                num_idxs=nb * 128, num_idxs_reg=nb * 128, elem_size=128,
                single_packet=False, queue_num=self.next_q())
            for k, (w, kind, j) in enumerate(blks):
                first, last = st.first_last[start + k]
                if first:
                    ps[w] = self.p_ps.tile([128, 128], f32, tag="seg", name="seg_ps")
                lhsT = self.ident_t[:] if kind == 0 else \
                    oh_t[:, (j - oh0) * 128:(j - oh0 + 1) * 128]
                nc.tensor.matmul(ps[w][:], lhsT, g_t[:, k, :],
                                 start=first, stop=last)
                if last:
                    fl = self.p_fl.tile([128, cols], out_dtype, tag=f"fl{cols}")
                    nc.scalar.activation(fl[:], ps[w][:, 0:cols], func,
                                         scale=scale_t[:, w:w + 1])
                    nc.sync.dma_start(dst_dram[128 * w:128 * (w + 1), 0:cols],
                                      fl[:])
                    del ps[w]
        # zero-fill windows with no entries
        ztile = self.z16_t if out_dtype == f16 else self.z32_t
        for w in range(st.nw):
            if not st.has_any[w]:
                nc.sync.dma_start(dst_dram[128 * w:128 * (w + 1), 0:cols],
                                  ztile[:, 0:cols])

    def emit_fused_stage(self, st: SegStage, xexpT_d, dvb_d, W_t, brow_t,
                         scale_t, dst_dram):
        """st1_0 with lin0 fused: rhs tiles computed inline from host-expanded
        transposed input (slab loads, no gather)."""
        nc, mybir = self.nc, self.mybir
        f32, f16 = mybir.dt.float32, mybir.dt.float16
        act = mybir.ActivationFunctionType
        oh_d = None
        if st.n_oh:
            oh_d = self.add_input(f"{st.name}_oh", [128, st.n_oh * 128], f16,
                                  st.oh_planes)
        ps = {}
        for start in range(0, st.nblocks, CH):
            nb = min(CH, st.nblocks - start)
            blks = st.blocks[start:start + nb]
            slab = self.p_gath.tile([128, nb * 128], f16, tag="xslab")
            nc.sync.dma_start(slab[:],
                              xexpT_d[:, start * 128:(start + nb) * 128])
            dvb_t = self.p_meta.tile([1, nb * 128], f16, tag="dvb")
            nc.sync.dma_start(dvb_t[:],
                              dvb_d[:, start * 128:(start + nb) * 128])
            ohs = [j for (_, kind, j) in blks if kind == 1]
            oh_t, oh0 = None, 0
            if ohs:
                oh0 = ohs[0]
                noh = ohs[-1] - oh0 + 1
                oh_t = self.p_meta.tile([128, noh * 128], f16, tag="oh")
                nc.sync.dma_start(oh_t[:],
                                  oh_d[:, oh0 * 128:(oh0 + noh) * 128])
            for k, (w, kind, j) in enumerate(blks):
                first, last = st.first_last[start + k]
                ps_lin = self.p_ps2.tile([128, 128], f32, tag="flin",
                                         name="flin_ps")
                nc.tensor.matmul(ps_lin[:], slab[:, k * 128:(k + 1) * 128],
                                 W_t[:], start=True, stop=False)
                nc.tensor.matmul(ps_lin[:], dvb_t[:, k * 128:(k + 1) * 128],
                                 brow_t[:], start=False, stop=True)
                fl_lin = self.p_lin.tile([128, 128], f16, tag="flin_s")
                nc.vector.tensor_copy(fl_lin[:], ps_lin[:])
                if first:
                    ps[w] = self.p_ps.tile([128, 128], f32, tag="seg",
                                           name="seg_ps")
                lhsT = self.ident_t[:] if kind == 0 else \
                    oh_t[:, (j - oh0) * 128:(j - oh0 + 1) * 128]
                nc.tensor.matmul(ps[w][:], lhsT, fl_lin[:],
                                 start=first, stop=last)
                if last:
                    fl = self.p_fl.tile([128, 128], f16, tag="fl128")
                    nc.scalar.activation(fl[:], ps[w][:], act.Copy,
                                         scale=scale_t[:, w:w + 1])
                    nc.sync.dma_start(dst_dram[128 * w:128 * (w + 1), :],
                                      fl[:])
                    del ps[w]
        ztile = self.z16_t
        for w in range(st.nw):
            if not st.has_any[w]:
                nc.sync.dma_start(dst_dram[128 * w:128 * (w + 1), :],
                                  ztile[:])

    def emit_linear(self, name, sources, Ws, bias_t, scale_t, dst_dram,
                    nchunks):
        """dst chunk = (sum_s srcT_chunk_s.T @ Ws[s] + bias) * scale.

        sources: list of ('hostT', dram [128, n]) | ('dramT', dram [n, 128])
                 | ('gatherT', dram, idx_dram)."""
        nc, mybir = self.nc, self.mybir
        f32, f16, i16 = mybir.dt.float32, mybir.dt.float16, mybir.dt.int16
        act = mybir.ActivationFunctionType
        gtiles = {}

        def srcT(si, i, spec):
            kind = spec[0]
            if kind == "hostT":
                t = self.p_lin.tile([128, 128], f16, tag="lt")
                nc.sync.dma_start(t[:], spec[1][:, 128 * i:128 * (i + 1)])
                return t[:]
            if kind == "dramT":
                t = self.p_lin.tile([128, 128], f16, tag="lt")
                nc.sync.dma_start(t[:], spec[1][128 * i:128 * (i + 1), :],
                                  transpose=True)
                return t[:]
            grp = i // GCH
            if (si, grp) not in gtiles:
                n_in = min(GCH, nchunks - grp * GCH)
                idx_t = self.p_meta.tile([128, n_in * 8], i16, tag="lidx")
                nc.sync.dma_start(
                    idx_t[:],
                    spec[2][:, grp * GCH * 8:(grp * GCH + n_in) * 8])
                g_t = self.p_gath.tile([128, 1, n_in * 128], f16, tag="lg")
                nc.gpsimd.dma_gather(
                    g_t[:], spec[1][:, :], idx_t[:],
                    num_idxs=n_in * 128, num_idxs_reg=n_in * 128,
                    elem_size=128, transpose=True,
                    single_packet=False, queue_num=self.next_q())
                gtiles[(si, grp)] = g_t
            return gtiles[(si, grp)][:, 0, 128 * (i % GCH):128 * (i % GCH + 1)]

        for i in range(nchunks):
            ps = self.p_ps2.tile([128, 128], f32, tag="lin")
            for si, spec in enumerate(sources):
                nc.tensor.matmul(ps[:], srcT(si, i, spec), Ws[si][:],
                                 start=(si == 0), stop=False)
            nc.tensor.matmul(ps[:], self.ones1_t[:], bias_t[:],
                             start=False, stop=True)
            fl = self.p_fl.tile([128, 128], f16, tag="lfl")
            nc.scalar.activation(fl[:], ps[:], act.Copy,
                                 scale=scale_t[:, i:i + 1])
            nc.sync.dma_start(dst_dram[128 * i:128 * (i + 1), :], fl[:])


# ---------------------------------------------------------------- main
def prepare(inputs):
    """Host-side schedule construction (numpy only, no device imports)."""
    H = [
        (np.asarray(inputs["H0_v"]).astype(np.int64),
         np.asarray(inputs["H0_e"]).astype(np.int64), N0, E0),
        (np.asarray(inputs["H1_v"]).astype(np.int64),
         np.asarray(inputs["H1_e"]).astype(np.int64), N1, E1),
        (np.asarray(inputs["H2_v"]).astype(np.int64),
         np.asarray(inputs["H2_e"]).astype(np.int64), N2, E2),
    ]
    assign0 = np.asarray(inputs["assign0"]).astype(np.int64)
    assign1 = np.asarray(inputs["assign1"]).astype(np.int64)

    n0l, n1l, n2l = _pad_local(N0), _pad_local(N1), _pad_local(N2)
    e0p, e1p, e2p = _pad128(E0), _pad128(E1), _pad128(E2)
    P = {"n0l": n0l, "n1l": n1l, "n2l": n2l,
         "e0p": e0p, "e1p": e1p, "e2p": e2p}

    def lap(lv, nloc):
        vi, ei, n, e = H[lv]
        dv_is, de_i = _degrees(vi, ei, n, e)
        owner, slot = vi % C, vi // C
        s1s, s1t, s2s, s2t = [], [], [], []
        for c in range(C):
            m = owner == c
            s1s.append(slot[m]); s1t.append(ei[m])
            s2s.append(ei[m]); s2t.append(slot[m])
        ep = _pad128(e)
        st1 = SegStage(f"l{lv}s1", s1s, s1t, nloc, ep)
        st2 = SegStage(f"l{lv}s2", s2s, s2t, ep, nloc)
        # scale arrays: [128, nw] column-per-window (f32)
        de_pad = np.zeros(ep, np.float32); de_pad[:e] = de_i[:e]
        sc1 = np.ascontiguousarray(de_pad.reshape(-1, 128).T)
        sc2 = []
        for c in range(C):
            g = np.arange(c, n, C)
            dv_loc = np.zeros(nloc, np.float32)
            dv_loc[: len(g)] = dv_is[g]
            sc2.append(np.ascontiguousarray(dv_loc.reshape(-1, 128).T))
        return st1, st2, sc1, sc2

    st1_0, st2_0, de0_sc, dv0_sc = lap(0, n0l)
    st1_1, st2_1, de1_sc, dv1_sc = lap(1, n1l)
    st1_2, st2_2, de2_sc, dv2_sc = lap(2, n2l)

    def pool(name, assign, nfine, ncoarse, ncl):
        cnt = np.bincount(assign, minlength=ncoarse).astype(np.float32)
        inv = np.where(cnt > 0, 1.0 / cnt, 0.0).astype(np.float32)
        srcs, tgts = [], []
        for c in range(C):
            g = np.arange(c, nfine, C)
            a = assign[g]
            srcs.append(g // C)
            tgts.append((a % C) * ncl + a // C)
        nfl = _pad_local(nfine)
        st = SegStage(name, srcs, tgts, nfl, C * ncl)
        # inv over P rows: r -> cluster a = (r % ncl)*C + r//ncl
        rows = np.arange(C * ncl)
        a = (rows % ncl) * C + rows // ncl
        sc = np.where(a < ncoarse, inv[np.minimum(a, ncoarse - 1)], 0.0)
        sc = np.ascontiguousarray(sc.astype(np.float32).reshape(-1, 128).T)
        return st, sc

    pool0, inv1_sc = pool("pool0", assign0, N0, N1, n1l)
    pool1, inv2_sc = pool("pool1", assign1, N1, N2, n2l)

    def unpool_idx(assign, nfine, ncl, nchunks):
        idxs = []
        for c in range(C):
            a = assign[np.arange(c, nfine, C)]
            idxs.append((a % C) * ncl + a // C)
        return _lin_gather_idx(idxs, nchunks)

    up1_arrs, up1_planes = unpool_idx(assign1, N1, n2l, n1l // 128)
    up0_arrs, up0_planes = unpool_idx(assign0, N0, n1l, n0l // 128)

    # host-expanded X in st1_0 slot order (fused lin0+stage1, no gather):
    # col s of xexpT = dv0[v(s)] * X[v(s)], transposed [128, L]; dvb = dv0[v(s)]
    X = np.asarray(inputs["X"], np.float32)
    vi0 = H[0][0]
    dv_is0, _ = _degrees(H[0][0], H[0][1], N0, E0)
    xexpT, dvb = [], []
    for c in range(C):
        g = np.arange(c, N0, C)
        xl = np.zeros((n0l + 128, D_IN), np.float32)
        xl[: len(g)] = X[g] * dv_is0[g][:, None]
        dvl = np.zeros(n0l + 128, np.float32)
        dvl[: len(g)] = dv_is0[g]
        idx = st1_0.idx_arrs[c]
        xexpT.append(np.ascontiguousarray(xl[idx].T.astype(np.float16)))
        dvb.append(np.ascontiguousarray(dvl[idx].astype(np.float16)
                                        .reshape(1, -1)))

    W = {k: np.asarray(inputs[k], np.float32) for k in
         ("W0", "W1", "W2", "W3", "W4", "b0", "b1", "b2", "b3", "b4")}
    return dict(P=P, stages=dict(
        st1_0=st1_0, st2_0=st2_0, st1_1=st1_1, st2_1=st2_1,
        st1_2=st1_2, st2_2=st2_2, pool0=pool0, pool1=pool1),
        scales=dict(de0=de0_sc, dv0=dv0_sc, de1=de1_sc, dv1=dv1_sc,
                    de2=de2_sc, dv2=dv2_sc, inv1=inv1_sc, inv2=inv2_sc),
        up0=(up0_arrs, up0_planes), up1=(up1_arrs, up1_planes),
        xexpT=xexpT, dvb=dvb, W=W)


def emulate(prep, inputs):
    """Pure-numpy replay of the device schedule (fp32 math)."""
    P = prep["P"]; S = prep["stages"]; SC = prep["scales"]; W = prep["W"]
    n0l, n1l, n2l = P["n0l"], P["n1l"], P["n2l"]
    e0p, e1p, e2p = P["e0p"], P["e1p"], P["e2p"]

    def f16(a):
        return a.astype(np.float16).astype(np.float32)

    def lin(sources_T, Ws, b, scale_cols, rows):
        outs = []
        for c in range(C):
            acc = sum(sT.T @ w for sT, w in zip(sources_T[c], Ws)) + b
            sc = scale_cols[c] if isinstance(scale_cols, list) else scale_cols
            acc = acc * sc.T.reshape(-1)[:rows, None]
            outs.append(f16(acc))
        return outs

    def with_z(tabs, pad_rows=128):
        return [np.vstack([t, np.zeros((pad_rows, t.shape[1]), np.float32)])
                for t in tabs]

    # fused lin0+st1_0: per-slot rows from xexpT, same block schedule
    st10 = S["st1_0"]
    W0q = f16(W["W0"])
    Y0p = []
    for c in range(C):
        rows = f16(prep["xexpT"][c].astype(np.float32).T @ W0q
                   + prep["dvb"][c].astype(np.float32).T * W["b0"])
        scflat = SC["de0"].T.reshape(-1)
        out = np.zeros((st10.nw * 128, 128), np.float32)
        ps = {}
        for i, (w, kind, j) in enumerate(st10.blocks):
            g = rows[i * 128:(i + 1) * 128]
            if w not in ps:
                ps[w] = np.zeros((128, 128), np.float32)
            if kind == 0:
                ps[w] += g
            else:
                ps[w] += st10.oh_arrs[c][j].astype(np.float32).T @ g
            if st10.first_last[i][1]:
                out[w * 128:(w + 1) * 128] = \
                    ps.pop(w) * scflat[w * 128:(w + 1) * 128, None]
        Y0p.append(out)
    Y0f = [f16(sum(Y0p))] * C
    h0 = S["st2_0"].emulate(with_z(Y0f), n0l, SC["dv0"], True)
    h0 = [f16(t) for t in h0]
    P1p = S["pool0"].emulate(with_z(h0), C * n1l, [SC["inv1"]] * C, False)
    P1s_full = sum(P1p)
    P1s = [f16(P1s_full[c * n1l:(c + 1) * n1l]) for c in range(C)]
    T1 = lin([[P1s[c].T] for c in range(C)], [f16(W["W1"])], W["b1"],
             SC["dv1"], n1l)
    Y1p = S["st1_1"].emulate(with_z(T1), e1p, [SC["de1"]] * C, False)
    Y1f = [f16(sum(Y1p))] * C
    h1 = S["st2_1"].emulate(with_z(Y1f), n1l, SC["dv1"], True)
    h1 = [f16(t) for t in h1]
    P2p = S["pool1"].emulate(with_z(h1), C * n2l, [SC["inv2"]] * C, False)
    P2s_full = sum(P2p)
    P2s = [f16(P2s_full[c * n2l:(c + 1) * n2l]) for c in range(C)]
    T2 = lin([[P2s[c].T] for c in range(C)], [f16(W["W2"])], W["b2"],
             SC["dv2"], n2l)
    Y2p = S["st1_2"].emulate(with_z(T2), e2p, [SC["de2"]] * C, False)
    Y2f = [f16(sum(Y2p))] * C
    Xc2 = S["st2_2"].emulate(with_z(Y2f), n2l, SC["dv2"], True)
    Xc2 = [f16(t) for t in Xc2]
    Xc2f = np.vstack(Xc2)  # [C*n2l, 128]
    Xc2fz = np.vstack([Xc2f, np.zeros((128, 128), np.float32)])
    up0_arrs, _ = prep["up0"]; up1_arrs, _ = prep["up1"]
    W3 = f16(W["W3"]); W4 = f16(W["W4"])
    T3 = lin([[Xc2fz[up1_arrs[c]].T, h1[c].T] for c in range(C)],
             [W3[:128], W3[128:]], W["b3"], SC["dv1"], n1l)
    Y3p = S["st1_1"].emulate(with_z(T3), e1p, [SC["de1"]] * C, False)
    Y3f = [f16(sum(Y3p))] * C
    Xu1 = S["st2_1"].emulate(with_z(Y3f), n1l, SC["dv1"], True)
    Xu1 = [f16(t) for t in Xu1]
    Xuf = np.vstack(Xu1)
    Xufz = np.vstack([Xuf, np.zeros((128, 128), np.float32)])
    W4p = np.zeros((256, 128), np.float32); W4p[:, :64] = W4
    b4p = np.zeros(128, np.float32); b4p[:64] = W["b4"]
    T4 = lin([[Xufz[up0_arrs[c]].T, h0[c].T] for c in range(C)],
             [W4p[:128], W4p[128:]], b4p, SC["dv0"], n0l)
    Y4p = S["st1_0"].emulate(with_z(T4), e0p, [SC["de0"]] * C, False)
    Y4f = [f16(sum(Y4p))] * C
    outs = S["st2_0"].emulate(with_z(Y4f), n0l, SC["dv0"], False, cols=64)
    out = np.empty((N0, D_OUT), np.float32)
    for c in range(C):
        n = len(range(c, N0, C))
        out[c::C] = outs[c][:n]
    return out


def build(prep):
    import concourse.bass as bass  # noqa: F401
    import concourse.tile as tile
    from concourse import bacc, mybir
    from contextlib import ExitStack

    P = prep["P"]; S = prep["stages"]; SC = prep["scales"]; W = prep["W"]
    n0l, n1l, n2l = P["n0l"], P["n1l"], P["n2l"]
    e0p, e1p, e2p = P["e0p"], P["e1p"], P["e2p"]

    nc = bacc.Bacc("TRN2", target_bir_lowering=False, debug=False,
                   num_devices=C, num_swdge_queues=NQ)
    f32, f16, i16 = mybir.dt.float32, mybir.dt.float16, mybir.dt.int16
    B = Builder(nc, mybir)

    def dram(name, rows, d=128, dt=None, shared=False):
        return nc.dram_tensor(name, [rows, d], dt or f16,
                              addr_space="Shared" if shared else "Local")

    # tables (+128 zero window on every gather source)
    Y0p = dram("Y0p", e0p); Y0f = dram("Y0f", e0p + 128, shared=True)
    h0 = dram("h0", n0l + 128)
    P1p = dram("P1p", C * n1l); P1s = dram("P1s", n1l)
    T1 = dram("T1", n1l + 128)
    Y1p = dram("Y1p", e1p); Y1f = dram("Y1f", e1p + 128, shared=True)
    h1 = dram("h1", n1l + 128)
    P2p = dram("P2p", C * n2l); P2s = dram("P2s", n2l)
    T2 = dram("T2", n2l + 128)
    Y2p = dram("Y2p", e2p); Y2f = dram("Y2f", e2p + 128, shared=True)
    Xc2 = dram("Xc2", n2l); Xc2f = dram("Xc2f", C * n2l + 128, shared=True)
    T3 = dram("T3", n1l + 128)
    Y3p = dram("Y3p", e1p); Y3f = dram("Y3f", e1p + 128, shared=True)
    Xu1 = dram("Xu1", n1l); Xuf = dram("Xuf", C * n1l + 128, shared=True)
    T4 = dram("T4", n0l + 128)
    Y4p = dram("Y4p", e0p); Y4f = dram("Y4f", e0p + 128, shared=True)
    out_d = nc.dram_tensor("out", [n0l, D_OUT], f32, kind="ExternalOutput")

    L10 = S["st1_0"].L
    xexpT_d = B.add_input("xexpT", [128, L10], f16, prep["xexpT"])
    dvb_d = B.add_input("dvb", [1, L10], f16, prep["dvb"])
    up0_d = B.add_input("up0_idx", list(prep["up0"][1][0].shape), i16,
                        prep["up0"][1])
    up1_d = B.add_input("up1_idx", list(prep["up1"][1][0].shape), i16,
                        prep["up1"][1])

    rg = [list(range(C))]

    def coll(kind, src_ap, dst_ap):
        op = mybir.AluOpType.bypass if kind == "AllGather" else \
            mybir.AluOpType.add
        nc.gpsimd.collective_compute(
            kind, op, replica_groups=rg,
            ins=[src_ap.opt()], outs=[dst_ap.opt()])

    with ExitStack() as ctx:
        tc = ctx.enter_context(tile.TileContext(nc))
        B.tc = tc
        B.setup_pools(ctx)
        B.setup_consts()
        W0t = B.const_mat("w0", W["W0"].astype(np.float16))
        W1t = B.const_mat("w1", W["W1"].astype(np.float16))
        W2t = B.const_mat("w2", W["W2"].astype(np.float16))
        W3a = B.const_mat("w3a", W["W3"][:128].astype(np.float16))
        W3b = B.const_mat("w3b", W["W3"][128:].astype(np.float16))
        W4pad = np.zeros((256, 128), np.float16)
        W4pad[:, :64] = W["W4"].astype(np.float16)
        W4a = B.const_mat("w4a", W4pad[:128])
        W4b = B.const_mat("w4b", W4pad[128:])
        b4pad = np.zeros(128, np.float32); b4pad[:64] = W["b4"]
        bts = {}
        for k, v in (("b0", W["b0"]), ("b1", W["b1"]), ("b2", W["b2"]),
                     ("b3", W["b3"]), ("b4", b4pad)):
            bts[k] = B.const_mat(k, v.astype(np.float16).reshape(1, 128))
        scs = {}
        for k in ("de0", "de1", "de2", "inv1", "inv2"):
            scs[k] = B.const_mat("sc_" + k, SC[k])
        for k in ("dv0", "dv1", "dv2"):
            scs[k] = B.const_mat("sc_" + k, SC[k])

        # zero windows for all gather-source tables
        for t, rows in ((T1, n1l), (T2, n2l), (T3, n1l),
                        (T4, n0l), (Y0f, e0p), (Y1f, e1p), (Y2f, e2p),
                        (Y3f, e1p), (Y4f, e0p), (h0, n0l), (h1, n1l),
                        (Xc2f, C * n2l), (Xuf, C * n1l)):
            nc.sync.dma_start(t[rows:rows + 128, :], B.z16_t[:])

        B.emit_fused_stage(S["st1_0"], xexpT_d, dvb_d, W0t, bts["b0"],
                           scs["de0"], Y0p)
        coll("AllReduce", Y0p.ap(), Y0f[0:e0p, :])
        B.emit_stage(S["st2_0"], Y0f, h0, scs["dv0"], True)
        B.emit_stage(S["pool0"], h0, P1p, scs["inv1"], False)
        coll("ReduceScatter", P1p.ap(), P1s.ap())
        B.emit_linear("lin1", [("dramT", P1s)], [W1t], bts["b1"],
                      scs["dv1"], T1, n1l // 128)
        B.emit_stage(S["st1_1"], T1, Y1p, scs["de1"], False)
        coll("AllReduce", Y1p.ap(), Y1f[0:e1p, :])
        B.emit_stage(S["st2_1"], Y1f, h1, scs["dv1"], True)
        B.emit_stage(S["pool1"], h1, P2p, scs["inv2"], False)
        coll("ReduceScatter", P2p.ap(), P2s.ap())
        B.emit_linear("lin2", [("dramT", P2s)], [W2t], bts["b2"],
                      scs["dv2"], T2, n2l // 128)
        B.emit_stage(S["st1_2"], T2, Y2p, scs["de2"], False)
        coll("AllReduce", Y2p.ap(), Y2f[0:e2p, :])
        B.emit_stage(S["st2_2"], Y2f, Xc2, scs["dv2"], True)
        coll("AllGather", Xc2.ap(), Xc2f[0:C * n2l, :])
        B.emit_linear("lin3", [("gatherT", Xc2f, up1_d), ("dramT", h1)],
                      [W3a, W3b], bts["b3"], scs["dv1"], T3, n1l // 128)
        B.emit_stage(S["st1_1"].renamed("l1bs1"), T3, Y3p, scs["de1"], False)
        coll("AllReduce", Y3p.ap(), Y3f[0:e1p, :])
        B.emit_stage(S["st2_1"].renamed("l1bs2"), Y3f, Xu1, scs["dv1"], True)
        coll("AllGather", Xu1.ap(), Xuf[0:C * n1l, :])
        B.emit_linear("lin4", [("gatherT", Xuf, up0_d), ("dramT", h0)],
                      [W4a, W4b], bts["b4"], scs["dv0"], T4, n0l // 128)
        B.emit_stage(S["st1_0"].renamed("l0bs1"), T4, Y4p, scs["de0"], False)
        coll("AllReduce", Y4p.ap(), Y4f[0:e0p, :])
        B.emit_stage(S["st2_0"].renamed("l0bs2"), Y4f, out_d, scs["dv0"],
                     False, out_dtype=f32, cols=D_OUT)
    nc.compile()

    in_maps = []
    for c in range(C):
        m = {}
        for name, arrs in B.inputs.items():
            m[name] = arrs[c] if isinstance(arrs, list) else arrs
        in_maps.append(m)
    return nc, in_maps


LAST_EXEC_NS = None


def _install_ntff_hook():
    import contextlib, ctypes, types
    try:
        from antenv import axon_hooks  # noqa: F401
        return
    except ImportError:
        pass
    import antenv
    so_path = os.environ.get("PJRT_LIBRARY_PATH", "/opt/axon/libaxon_pjrt.so")
    try:
        lib = ctypes.CDLL(so_path)
    except OSError:
        lib = None
    hook = None
    if lib is not None and hasattr(lib, "axon_start_nrt_profile"):
        lib.axon_start_nrt_profile.argtypes = [
            ctypes.POINTER(ctypes.c_int64), ctypes.c_size_t]
        lib.axon_start_nrt_profile.restype = ctypes.c_int64
        lib.axon_stop_nrt_profile.argtypes = [ctypes.c_char_p]
        lib.axon_stop_nrt_profile.restype = ctypes.c_int64

        @contextlib.contextmanager
        def hook(output_dir, device_ids):
            import jax
            jax.devices()
            if device_ids:
                ids = (ctypes.c_int64 * len(device_ids))(*device_ids)
                rc = lib.axon_start_nrt_profile(ids, len(device_ids))
            else:
                rc = lib.axon_start_nrt_profile(None, 0)
            if rc != 0:
                raise RuntimeError(f"axon_start_nrt_profile rc={rc}")
            try:
                yield
            finally:
                lib.axon_stop_nrt_profile(str(output_dir).encode())

    mod = types.ModuleType("antenv.axon_hooks")
    mod._hook = hook
    mod.get_axon_ntff_profile_hook = lambda: mod._hook

    def _set(h):
        mod._hook = h
    mod.set_axon_ntff_profile_hook = _set
    sys.modules["antenv.axon_hooks"] = mod
    antenv.axon_hooks = mod


def kernel(**inputs):
    global LAST_EXEC_NS
    prep = prepare(inputs)
    if os.environ.get("HGNN_EMULATE", "0") == "1":
        return emulate(prep, inputs)
    trace = os.environ.get("HGNN_TRACE", "0") == "1"
    if trace:
        _install_ntff_hook()
    nc, in_maps = build(prep)
    from concourse.bass_utils import run_bass_kernel_spmd
    res = run_bass_kernel_spmd(nc, in_maps, core_ids=list(range(C)),
                               trace=trace)
    LAST_EXEC_NS = res.exec_time_ns
    out = np.empty((N0, D_OUT), np.float32)
    for c in range(C):
        n = len(range(c, N0, C))
        out[c::C] = res.results[c]["out"][:n]
    return out
